# revision 27
# baseline (speedup 1.0000x reference)
"""
Self-attention (SAGAN-style) Bass kernel for Trainium2, 8 NeuronCores.

Reference computation (per batch b, X = x[b] reshaped [C=256, N=4096]):
    q = W1 X + b1          [D=32, N]
    k = W2 X + b2          [D=32, N]
    v = W3 X + b3          [C, N]
    E[n, m]   = q[:, n] . k[:, m]
    A         = softmax over m of E
    out[c, n] = sum_m v[c, m] A[n, m]  + X[c, n]

Sharding: 8 cores = 4 batches x 2 query-halves. Each core gets the full
X for its batch (needed for K/V over all N) but computes only its half of
the query rows -> no inter-core communication at all.

Device algorithm (per core, NQ = 2048 query columns):
  - Q' = W1 X[:, 0:NQ], K' = W2 X, each replicated 4x along partitions
    (host pre-tiles the weights) so the K=32 energy matmuls can use
    PE row-tiling.
  - V_T[m, c] = (X^T W3^T)[m, c] computed natively (no transposes), with
    an appended constant ones column: V_aug = [V_T | 1 | 0pad]. The V bias
    b3 drops out of the matmuls entirely: its contribution to out_T is
    Z[n]*b3[c], which after the /Z normalization is just +b3 -- folded
    into the host-prepared residual xqT.
  - E^T[m, n] = K'^T Q' with m on partitions -> exp(E - 25) without any
    per-row max subtraction (softmax is shift-invariant; the constant
    rides the ACT affine and keeps fp32 safe up to |E| ~ 113) -> P^T. The two K=32 energy
    matmuls of each group run concurrently in separate PE row-bands
    (tile_position), and exp is split in two so PV can start after the
    first half.
  - out_T[n, c], Z[n] = P^T.T @ V_aug accumulated over all m in PSUM:
    the ones column yields the softmax denominator for free.
  - out = out_T[:, 0:256] * (1/Z) + (X_q^T + b3)  (one fused vector op),
    DMA out in [n, c] layout; the host transposes back when unsharding.

All matmul operands are stored as bf16 (x, weights, Q', K', P, V_aug):
the PE streams 1 column/cycle either way, but bf16 halves the
LDWEIGHTS cost via FWL (fast weight load) — and on this toolchain the
per-matmul weight load is fully serialized with the matmul (measured
156 ns for LDW+MM at 258 free-dim), so LDW time is on the critical
path everywhere. PSUM accumulation stays fp32; the residual/normalize
path (xqT, biases, 1/Z) stays fp32. Energy groups alternate PE row
bands (g even: rows 0-63, g odd: 64-127) so consecutive groups'
LDWEIGHTS can pull ahead and their matmuls run concurrently.
Measured on HW (test.py loop-slope): 150278 ns (fp32r baseline)
-> 134466 (bf16 operands, merged exp, band alternation)
-> 132468 (fused prep, bf16 x/weights)
-> 128378 (For_i staggered_reset + PE branch-prefetch hints).
PE-busy floor model: PV 512x(107.5 stream + 53 LDW + 4 NX) = 84us,
energy ~14.5us (2-band concurrent), prep ~17us -> ~119us; the wall
is PE-bound, ACT exp is ~77us busy and off the critical path.
"""

import numpy as np

E_PACK = True  # row-tile the K=32 energy matmuls via tile_position
HEAD_SPLIT = True  # 256-col first chunk: first matmul starts earlier
PREP_FUSE = True  # weave chunk-0 attention into the prep phase
OUT_ONE_DMA = True  # one output DMA per n-chunk instead of per n-sub
E_BF16 = True  # Q'/K' in bf16: FWL fast weight loads on energy matmuls
VT_STT = True  # V bias via one DVE scalar_tensor_tensor instead of K=1 matmuls
EXP_SPLIT = False  # two ACT instructions per group: PV can start after the first half
ABL_EXP_HALF = False  # timing ablation: exp over half the tile (WRONG results)
ABL_PV_HALF = False  # timing ablation: skip half the PV matmuls (WRONG results)
MG = 2  # m-tiles per exp group (<=4; MG concurrent PE row-bands). MG=4 with
# NCHW=256 is ILLEGAL: the band matmuls would write 256-col halves of the
# same PSUM bank concurrently -> fatal bank write collision on HW.
NCHW = 512  # n-chunk width (columns of Q processed per oT accumulation pass)
E_BUFS = 2  # psum double/triple buffering for energy tiles
PV_BF16 = True  # store P and V_aug as bf16: FWL fast weight load on PV matmuls
E_BAND_ALT = True  # alternate PE row-bands across energy groups: LDW overlaps
SPLIT_LDW = False  # bacc already auto-splits LDW from matmuls; extras load twice
X_BF16 = True  # xf/wpack in bf16: FWL on V-prep/KQ-proj stationaries, half DMA
LOOP_HINTS = True  # For_i branch prefetch hints (body >256 PE instructions)

B = 4
C = 256
D = 32
N = 4096
NQ = N // 2  # query columns per core
N_CORES = 8

_CACHE = {}


def _patch_tile_drain(tile_mod, mybir):
    """Walrus in this container rejects >2 sync waits on a Drain
    instruction ("Too many sync wait commands"). Split the TileContext
    tail-drain waits into single-wait NOPs instead."""
    if getattr(tile_mod.TileContext, "_drain_patched", False):
        return

    def _drain_and_barrier(self, tick_clock, wait_clock):
        from concourse.tile import ScopedClock

        drain_inst = self.nc.sync.drain()
        wait_clock.add_sem_waits(
            drain_inst.ins, ScopedClock({None: tick_clock.global_clock})
        )
        si = drain_inst.ins.sync_info
        if si is not None and len(si.on_wait) > 1:
            waits = list(si.on_wait)
            # Engine-owned sems are re-synchronized by the all_engine_barrier
            # below; only DMA-queue sems (no engine) must be awaited here.
            dma_waits = [w for w in waits if "DMA" in (w.ant_name or "")]
            eng_waits = [w for w in waits if "DMA" not in (w.ant_name or "")]
            keep = dma_waits if dma_waits else waits
            drain_inst.ins.sync_info = mybir.SyncInfo(
                on_wait=[keep[0]], on_update=[]
            )
            for w in keep[1:]:
                n = self.nc.sync.nop()
                n.ins.sync_info = mybir.SyncInfo(on_wait=[w], on_update=[])
        self.nc.all_engine_barrier()
        popped = self.nc._tile_sem_poison_stack.pop()
        assert popped is self._sem_poison
        self.nc.clear_and_free_semaphores(list(self.sems.allocated().values()))
        self.nc.all_engine_barrier()

    tile_mod.TileContext._drain_and_barrier = _drain_and_barrier
    tile_mod.TileContext._drain_patched = True


def _split_sync_waits(nc, mybir, max_waits=1):
    """This container's walrus rejects instructions carrying more than
    ~1 sync wait (e.g. Matmult/S3_LW, Drain/CTRL). Hoist extra waits onto
    same-engine NOPs placed immediately before the instruction."""
    ctr = [0]
    for fn in nc.m.functions:
        for blk in fn.blocks:
            new_insts = []
            for inst in blk.instructions:
                si = inst.sync_info
                if si is not None and len(si.on_wait) > max_waits:
                    waits = list(si.on_wait)
                    for w in waits[max_waits:]:
                        ctr[0] += 1
                        nop = mybir.InstNoOp(
                            name=f"WSPLIT-{ctr[0]}",
                            sync_info=mybir.SyncInfo(on_wait=[w], on_update=[]),
                            bass_nofuse=True,
                            engine=inst.engine,
                        )
                        new_insts.append(nop)
                    inst.sync_info = mybir.SyncInfo(
                        on_wait=waits[:max_waits], on_update=list(si.on_update)
                    )
                new_insts.append(inst)
            blk.instructions = new_insts


def _build_graph(repeat=1, loop_n=1):
    import sys

    if "/opt/trn_rl_repo" not in sys.path:
        sys.path.insert(0, "/opt/trn_rl_repo")
    import concourse.bass as bass
    import concourse.mybir as mybir
    import concourse.tile as tile

    _patch_tile_drain(tile, mybir)

    F32 = mybir.dt.float32
    F32R = mybir.dt.float32r
    BF16 = mybir.dt.bfloat16
    PVDT = BF16 if PV_BF16 else F32R
    EDT = BF16 if E_BF16 in (True, 1) else F32R
    KDT = BF16 if E_BF16 in (True, 1, 2) else F32R
    XDT = BF16 if X_BF16 else F32R
    Exp = mybir.ActivationFunctionType.Exp
    MULT = mybir.AluOpType.mult
    ADD = mybir.AluOpType.add

    _split_pairs = []  # names of InstMatmult with ldweights=False

    nc = bass.Bass()
    xf = nc.declare_dram_parameter("xf", [C, N], XDT, isOutput=False)
    xqT = nc.declare_dram_parameter("xqT", [NQ, C], F32, isOutput=False)
    wpack = nc.declare_dram_parameter("wpack", [C, 512], XDT, isOutput=False)
    bpack = nc.declare_dram_parameter("bpack", [128, 2], F32, isOutput=False)
    outp = nc.declare_dram_parameter("out", [NQ, C], F32, isOutput=True)

    MT = N // 128  # 32 m-tiles
    VW = C + 2  # 258: V_T plus ones column plus pad (fp32r needs even free dim)

    with tile.TileContext(nc) as tc:
        with tc.tile_pool(name="consts", bufs=1) as consts:
            # packed weights/biases first (few DMAs) so projections start fast
            # wp_s layout per c-half h (rows h*128..): [w1t | w2t | w3t] cols
            wp_s = consts.tile([128, 1024], XDT)
            nc.sync.dma_start(wp_s[:, 0:512], wpack[0:128, :])
            nc.sync.dma_start(wp_s[:, 512:1024], wpack[128:256, :])
            bp_s = consts.tile([128, 2], F32)
            w1t_s = [wp_s[:, 0:128], wp_s[:, 512:640]]  # per c-half
            w2t_s = [wp_s[:, 128:256], wp_s[:, 640:768]]
            w3t_s = [wp_s[:, 256:512], wp_s[:, 768:1024]]
            b1r_s = bp_s[:, 0:1]
            b2r_s = bp_s[:, 1:2]
            eshift_s = consts.tile([128, 1], F32)
            nc.vector.memset(eshift_s[:], -25.0)
            # xf in interleaved half-pairs so chunk-ch projections unblock
            # as soon as their slices land
            xf_s = consts.tile([128, 2 * N], XDT)
            if HEAD_SPLIT:
                for q in range(2):  # chunk 0 in 256-col quarters
                    nc.sync.dma_start(
                        xf_s[:, q * 256 : (q + 1) * 256],
                        xf[0:128, q * 256 : (q + 1) * 256],
                    )
                    nc.sync.dma_start(
                        xf_s[:, N + q * 256 : N + (q + 1) * 256],
                        xf[128:256, q * 256 : (q + 1) * 256],
                    )
                    if q == 0:
                        nc.sync.dma_start(bp_s[:], bpack[:])
            else:
                nc.sync.dma_start(bp_s[:], bpack[:])
            for ch in range(0 if not HEAD_SPLIT else 1, 8):
                nc.sync.dma_start(
                    xf_s[:, ch * 512 : (ch + 1) * 512],
                    xf[0:128, ch * 512 : (ch + 1) * 512],
                )
                nc.sync.dma_start(
                    xf_s[:, N + ch * 512 : N + (ch + 1) * 512],
                    xf[128:256, ch * 512 : (ch + 1) * 512],
                )
            xqT_s = consts.tile([128, (NQ // 128) * C], F32)
            for s in range(NQ // 128):
                nc.sync.dma_start(
                    xqT_s[:, s * C : (s + 1) * C], xqT[s * 128 : (s + 1) * 128, :]
                )

            qp_s = consts.tile([128, NQ], EDT)  # Q' replicated x4 on partitions
            kp_s = consts.tile([128, N], KDT)  # K' replicated x4
            vt_s = consts.tile([128, MT * VW], PVDT)  # [V_T | 1 | 0] per m-tile

            import contextlib

            loop_kw = (
                dict(
                    hint_engines=(mybir.EngineType.PE,),
                    staggered_reset=True,
                )
                if LOOP_HINTS
                else {}
            )
            loop_ctx = (
                tc.For_i(0, loop_n, 1, **loop_kw)
                if loop_n > 1
                else contextlib.nullcontext()
            )
            with loop_ctx:
              for _rep in range(repeat):
                # ---- compute body ----
                # Prep (K'/Q'/V_T projections) is emitted per 512-col chunk so
                # it tracks the xf DMA stream. With PREP_FUSE, chunk-0 of the
                # attention is woven between prep chunks (its group g needs
                # only the m-tiles prep chunk g//2 just produced), and the
                # prep PSUM tiles borrow the energy pool's slots so total
                # PSUM stays within 8 banks.
                NSUB = NCHW // 128
                NCH = NQ // NCHW
                NG = MT // MG
                import contextlib as _ctb

                with (
                    tc.tile_pool(name="ppool", bufs=4) as ppool,
                    tc.tile_pool(name="opool", bufs=4) as opool,
                    tc.tile_pool(name="zpool", bufs=6) as zpool,
                    _ctb.ExitStack() as _psum_stack,
                ):
                    _pools = {}

                    def open_po():
                        # 2 chunks of oT accumulators in flight (when they
                        # fit in PSUM next to the e tiles): the next chunk's
                        # PV matmuls start while the previous chunk's
                        # finalize is still reading its oT tiles.
                        po_bufs = 2 * NSUB if NSUB <= 2 else NSUB
                        _pools["po"] = _psum_stack.enter_context(
                            tc.tile_pool(name="po", bufs=po_bufs, space="PSUM")
                        )

                    def open_pe():
                        _pools["pe"] = _psum_stack.enter_context(
                            tc.tile_pool(name="pe", bufs=E_BUFS, space="PSUM")
                        )

                    def open_psum_pools():
                        open_pe()
                        open_po()

                    if _rep == 0:
                        vt_view = vt_s.rearrange("p (t w) -> p t w", w=VW)
                        mset_dt = BF16 if PV_BF16 else F32
                        nc.vector.memset(vt_view[:, :, C].bitcast(mset_dt), 1.0)
                        nc.vector.memset(vt_view[:, :, C + 1].bitcast(mset_dt), 0.0)

                    def _emit_v_tile(t, pool_v, v_tag):
                        ps = pool_v.tile(
                            [128, C], F32, name=f"r{_rep}psv{t}", tag=v_tag
                        )
                        nc.tensor.matmul(
                            ps[:],
                            xf_s[:, t * 128 : (t + 1) * 128],
                            w3t_s[0],
                            start=True,
                            stop=False,
                        )
                        nc.tensor.matmul(
                            ps[:],
                            xf_s[:, N + t * 128 : N + (t + 1) * 128],
                            w3t_s[1],
                            start=False,
                            stop=True,
                        )
                        nc.vector.tensor_copy(vt_s[:, t * VW : t * VW + C], ps[:])

                    def emit_proj_piece(pool, tag, w_s, b_s, dst, lo, w):
                        ps = pool.tile(
                            [128, w], F32, name=f"r{_rep}pp{lo}_{w}", tag=tag
                        )
                        nc.tensor.matmul(
                            ps[:], w_s[0], xf_s[:, lo : lo + w], start=True, stop=False
                        )
                        nc.tensor.matmul(
                            ps[:],
                            w_s[1],
                            xf_s[:, N + lo : N + lo + w],
                            start=False,
                            stop=True,
                        )
                        nc.vector.tensor_scalar_add(dst[:, lo : lo + w], ps[:], b_s)

                    def emit_K(ch, pool, tag):
                        emit_proj_piece(pool, tag, w2t_s, b2r_s, kp_s, ch * 512, 512)

                    def emit_Q(ch, pool, tag):
                        emit_proj_piece(pool, tag, w1t_s, b1r_s, qp_s, ch * 512, 512)

                    def emit_prep_chunk(ch, pool_kq, pool_v=None):
                        if pool_v is None:
                            pool_v = pool_kq
                        kq_tag = "e" if pool_kq is _pools.get("pe") else "psk"
                        v_tag = "e" if pool_v is _pools.get("pe") else "psv"
                        if ch == 0 and HEAD_SPLIT:
                            for q in range(2):
                                emit_proj_piece(
                                    pool_kq, kq_tag, w2t_s, b2r_s, kp_s, q * 256, 256
                                )
                            for q in range(2):
                                emit_proj_piece(
                                    pool_kq, kq_tag, w1t_s, b1r_s, qp_s, q * 256, 256
                                )
                            for t in range(4):
                                _emit_v_tile(t, pool_v, v_tag)
                            return
                        ps = pool_kq.tile(
                            [128, 512], F32, name=f"r{_rep}psk{ch}", tag=kq_tag
                        )
                        nc.tensor.matmul(
                            ps[:],
                            w2t_s[0],
                            xf_s[:, ch * 512 : (ch + 1) * 512],
                            start=True,
                            stop=False,
                        )
                        nc.tensor.matmul(
                            ps[:],
                            w2t_s[1],
                            xf_s[:, N + ch * 512 : N + (ch + 1) * 512],
                            start=False,
                            stop=True,
                        )
                        nc.vector.tensor_scalar_add(
                            kp_s[:, ch * 512 : (ch + 1) * 512], ps[:], b2r_s
                        )
                        if ch < NQ // 512:
                            ps = pool_kq.tile(
                                [128, 512], F32, name=f"r{_rep}psq{ch}", tag=kq_tag
                            )
                            nc.tensor.matmul(
                                ps[:],
                                w1t_s[0],
                                xf_s[:, ch * 512 : (ch + 1) * 512],
                                start=True,
                                stop=False,
                            )
                            nc.tensor.matmul(
                                ps[:],
                                w1t_s[1],
                                xf_s[:, N + ch * 512 : N + (ch + 1) * 512],
                                start=False,
                                stop=True,
                            )
                            nc.vector.tensor_scalar_add(
                                qp_s[:, ch * 512 : (ch + 1) * 512], ps[:], b1r_s
                            )
                        for t in range(ch * 4, ch * 4 + 4):
                            _emit_v_tile(t, pool_v, v_tag)

                    def emit_E(c, g, pool=None):
                        e = (pool or _pools["pe"]).tile(
                            [128, MG * NCHW], F32, name=f"r{_rep}e{c}_{g}", tag="e"
                        )
                        nbands = 4 // MG if E_BAND_ALT else 1
                        for i in range(MG):
                            mt = MG * g + i
                            band = (
                                32 * (MG * (g % nbands) + i) if E_PACK else 0
                            )
                            kw = dict(tile_position=(band, 0)) if E_PACK else {}
                            w_ap = kp_s[
                                band : band + 32, mt * 128 : (mt + 1) * 128
                            ]
                            if SPLIT_LDW and KDT is BF16:
                                nc.tensor.ldweights(w_ap, **kw)
                            mm = nc.tensor.matmul(
                                e[:, i * NCHW : (i + 1) * NCHW],
                                w_ap,
                                qp_s[band : band + 32, c * NCHW : (c + 1) * NCHW],
                                start=True,
                                stop=True,
                                **kw,
                            )
                            if SPLIT_LDW and KDT is BF16:
                                mm.ins.ldweights = False
                                _split_pairs.append(mm.ins.name)
                        return e

                    def make_oT(c):
                        return [
                            _pools["po"].tile(
                                [128, VW], F32, name=f"r{_rep}oT{c}_{s}", tag="oT"
                            )
                            for s in range(NSUB)
                        ]

                    def emit_group(c, g, e_cur, oT):
                        # exp(E - ESHIFT): softmax is shift-invariant; the
                        # constant rides the ACT instruction's free affine and
                        # moves the fp32 overflow ceiling from |E|~88 to ~113.
                        p = ppool.tile(
                            [128, MG * NCHW], PVDT, name=f"r{_rep}p{c}_{g}", tag="p"
                        )
                        if EXP_SPLIT:
                            h = MG * NCHW // 2
                            nc.scalar.activation(
                                p[:, 0:h], e_cur[:, 0:h], Exp, bias=eshift_s[:]
                            )
                            nc.scalar.activation(
                                p[:, h:], e_cur[:, h:], Exp, bias=eshift_s[:]
                            )
                        else:
                            nc.scalar.activation(
                                p[:], e_cur[:], Exp, bias=eshift_s[:]
                            )
                        for i in range(MG):
                            mt = MG * g + i
                            for s in range(NSUB):
                                nc.tensor.matmul(
                                    oT[s][:],
                                    p[:, i * NCHW + s * 128 : i * NCHW + (s + 1) * 128],
                                    vt_s[:, mt * VW : (mt + 1) * VW],
                                    start=(g == 0 and i == 0),
                                    stop=(g == NG - 1 and i == MG - 1),
                                )

                    def emit_finalize(c, oT, per_sub_dma=False):
                        otc = opool.tile(
                            [128, NSUB * C], F32, name=f"r{_rep}otc{c}", tag="ot"
                        )
                        for s in range(NSUB):
                            ns = c * NSUB + s
                            zinv = zpool.tile(
                                [128, 1], F32, name=f"r{_rep}zinv{c}_{s}", tag="zinv"
                            )
                            nc.vector.reciprocal(zinv[:], oT[s][:, C : C + 1])
                            nc.vector.scalar_tensor_tensor(
                                otc[:, s * C : (s + 1) * C],
                                oT[s][:, 0:C],
                                zinv[:],
                                xqT_s[:, ns * C : (ns + 1) * C],
                                MULT,
                                ADD,
                            )
                            if per_sub_dma:
                                nc.sync.dma_start(
                                    outp[ns * 128 : (ns + 1) * 128, :],
                                    otc[:, s * C : (s + 1) * C],
                                )
                        if not per_sub_dma:
                            dst = outp[
                                c * NSUB * 128 : (c + 1) * NSUB * 128, :
                            ].rearrange("(s p) c -> p s c", p=128)
                            nc.sync.dma_start(
                                dst, otc.rearrange("p (s c) -> p s c", c=C)
                            )

                    if PREP_FUSE:
                        open_po()
                        oT = make_oT(0)
                        with (
                            tc.tile_pool(name="pp", bufs=1, space="PSUM") as pp,
                            tc.tile_pool(name="pv", bufs=1, space="PSUM") as pv,
                            tc.tile_pool(name="pe1", bufs=1, space="PSUM") as pe1,
                        ):
                            gpc = 4 // MG  # chunk-0 groups per prep chunk
                            for ch in range(N // 512):
                                emit_K(ch, pp, "psk")
                                if ch == 0:
                                    emit_Q(0, pp, "psk")
                                _emit_v_tile(4 * ch + 0, pv, "psv")
                                _emit_v_tile(4 * ch + 1, pv, "psv")
                                if gpc == 2:
                                    g = 2 * ch
                                    emit_group(0, g, emit_E(0, g, pe1), oT)
                                _emit_v_tile(4 * ch + 2, pv, "psv")
                                _emit_v_tile(4 * ch + 3, pv, "psv")
                                if gpc == 2:
                                    g = 2 * ch + 1
                                else:
                                    g = ch
                                emit_group(0, g, emit_E(0, g, pe1), oT)
                                if 0 < ch < NQ // 512:
                                    emit_Q(ch, pp, "psk")
                        emit_finalize(0, oT)
                        open_pe()
                        # phase 2: remaining chunks, software-pipelined
                        steps = [(c, g) for c in range(1, NCH) for g in range(NG)]
                        e_cur = emit_E(*steps[0])
                        for k, (c, g) in enumerate(steps):
                            if g == 0:
                                oT = make_oT(c)
                            e_next = (
                                emit_E(*steps[k + 1]) if k + 1 < len(steps) else None
                            )
                            emit_group(c, g, e_cur, oT)
                            e_cur = e_next
                            if g == NG - 1:
                                emit_finalize(c, oT, per_sub_dma=(c == NCH - 1))
                    else:
                        with (
                            tc.tile_pool(name="pp", bufs=2, space="PSUM") as pp,
                            tc.tile_pool(name="pv", bufs=2, space="PSUM") as pv,
                        ):
                            for ch in range(N // 512):
                                emit_prep_chunk(ch, pp, pv)
                        open_psum_pools()
                        steps = [(c, g) for c in range(NCH) for g in range(NG)]
                        e_cur = emit_E(*steps[0])
                        for k, (c, g) in enumerate(steps):
                            if g == 0:
                                oT = make_oT(c)
                            e_next = (
                                emit_E(*steps[k + 1]) if k + 1 < len(steps) else None
                            )
                            emit_group(c, g, e_cur, oT)
                            e_cur = e_next
                            if g == NG - 1:
                                emit_finalize(c, oT, per_sub_dma=(c == NCH - 1))

    _verify_split_ldw(nc, mybir, set(_split_pairs))
    _split_sync_waits(nc, mybir)
    return nc


def _verify_split_ldw(nc, mybir, pair_names):
    """A matmul with ldweights=False computes with whatever the PE array
    holds; ensure the immediately-preceding weight-touching PE instruction
    is its own InstLdweights (same weights AP). If the Tile scheduler
    moved anything in between, restore the self-loading mode."""
    if not pair_names:
        return
    refused = 0
    for fn in nc.m.functions:
        for blk in fn.blocks:
            last_w = None  # (kind, weights_ap_repr)
            for inst in blk.instructions:
                if isinstance(inst, mybir.InstLdweights):
                    last_w = ("ldw", repr(inst.ins[0]))
                elif isinstance(inst, mybir.InstMatmult):
                    if inst.name in pair_names:
                        ok = (
                            last_w is not None
                            and last_w[0] == "ldw"
                            and last_w[1] == repr(inst.ins[1])
                        )
                        if not ok:
                            inst.ldweights = True
                            refused += 1
                    last_w = ("mm", None)
    if refused:
        import logging

        logging.getLogger(__name__).warning(
            f"_verify_split_ldw: re-fused {refused} matmuls"
        )


def _get_graph(repeat=1, loop_n=1):
    key = f"nc{repeat}_{loop_n}"
    if key not in _CACHE:
        _CACHE[key] = _build_graph(repeat, loop_n)
    return _CACHE[key]


def _host_inputs(x, w1, b1, w2, b2, w3, b3):
    x = np.asarray(x, dtype=np.float32)
    xf = x.reshape(B, C, N)
    w1t = np.tile(np.asarray(w1, np.float32).T, (1, 4))  # [256, 128]
    w2t = np.tile(np.asarray(w2, np.float32).T, (1, 4))
    w3t = np.ascontiguousarray(np.asarray(w3, np.float32).T)  # [256, 256]
    wpack = np.concatenate([w1t, w2t, w3t], axis=1)  # [256, 512]
    if X_BF16:
        import ml_dtypes

        wpack = wpack.astype(ml_dtypes.bfloat16)
    bpack = np.stack(
        [np.tile(np.asarray(b1, np.float32), 4), np.tile(np.asarray(b2, np.float32), 4)],
        axis=1,
    )  # [128, 2]
    b3f = np.asarray(b3, np.float32)

    in_maps = []
    for core in range(N_CORES):
        b, half = divmod(core, 2)
        n0 = half * NQ
        xb = xf[b]
        # roll so this core's query range sits at columns 0:NQ; K/V sums
        # over m are permutation-invariant so the roll is harmless.
        x_roll = np.ascontiguousarray(np.roll(xb, -n0, axis=1))
        if X_BF16:
            import ml_dtypes

            x_roll = x_roll.astype(ml_dtypes.bfloat16)
        xqTc = np.ascontiguousarray(xb[:, n0 : n0 + NQ].T + b3f[None, :])
        in_maps.append(
            {"xf": x_roll, "xqT": xqTc, "wpack": wpack, "bpack": bpack}
        )
    return in_maps


def kernel(x, w1, b1, w2, b2, w3, b3, _trace=False, _repeat=1):
    import sys

    if "/opt/trn_rl_repo" not in sys.path:
        sys.path.insert(0, "/opt/trn_rl_repo")
    from concourse.bass_utils import run_bass_kernel_spmd

    nc = _get_graph(_repeat)
    in_maps = _host_inputs(x, w1, b1, w2, b2, w3, b3)
    res = run_bass_kernel_spmd(nc, in_maps, list(range(N_CORES)), trace=_trace)
    _CACHE["last_result"] = res

    out = np.empty((B, C, N), np.float32)
    for core in range(N_CORES):
        b, half = divmod(core, 2)
        n0 = half * NQ
        out[b][:, n0 : n0 + NQ] = res.results[core]["out"].T
    return out.reshape(B, C, 64, 64)



# revision 32
# speedup vs baseline: 1.1336x; 1.1336x over previous
"""
Self-attention (SAGAN-style) Bass kernel for Trainium2, 8 NeuronCores.

Reference computation (per batch b, X = x[b] reshaped [C=256, N=4096]):
    q = W1 X + b1          [D=32, N]
    k = W2 X + b2          [D=32, N]
    v = W3 X + b3          [C, N]
    E[n, m]   = q[:, n] . k[:, m]
    A         = softmax over m of E
    out[c, n] = sum_m v[c, m] A[n, m]  + X[c, n]

Sharding: 8 cores = 4 batches x 2 query-halves. Each core gets the full
X for its batch (needed for K/V over all N) but computes only its half of
the query rows -> no inter-core communication at all.

Device algorithm (per core, NQ = 2048 query columns):
  - Q' = W1 X[:, 0:NQ], K' = W2 X, each replicated 4x along partitions
    (host pre-tiles the weights) so the K=32 energy matmuls can use
    PE row-tiling.
  - V_T[m, c] = (X^T W3^T)[m, c] computed natively (no transposes), with
    an appended constant ones column: V_aug = [V_T | 1 | 0pad]. The V bias
    b3 drops out of the matmuls entirely: its contribution to out_T is
    Z[n]*b3[c], which after the /Z normalization is just +b3 -- folded
    into the host-prepared residual xqT.
  - E^T[m, n] = K'^T Q' with m on partitions -> exp(E - 25) without any
    per-row max subtraction (softmax is shift-invariant; the constant
    rides the ACT affine and keeps fp32 safe up to |E| ~ 113) -> P^T. The two K=32 energy
    matmuls of each group run concurrently in separate PE row-bands
    (tile_position), and exp is split in two so PV can start after the
    first half.
  - out_T[n, c], Z[n] = P^T.T @ V_aug accumulated over all m in PSUM:
    the ones column yields the softmax denominator for free.
  - out = out_T[:, 0:256] * (1/Z) + (X_q^T + b3)  (one fused vector op),
    DMA out in [n, c] layout; the host transposes back when unsharding.

All matmul operands are stored as bf16 (x, weights, Q', K', P, V_aug):
the PE streams 1 column/cycle either way, but bf16 halves the
LDWEIGHTS cost via FWL (fast weight load) — and on this toolchain the
per-matmul weight load is fully serialized with the matmul (measured
156 ns for LDW+MM at 258 free-dim), so LDW time is on the critical
path everywhere. PSUM accumulation stays fp32; the residual/normalize
path (xqT, biases, 1/Z) stays fp32. Energy groups alternate PE row
bands (g even: rows 0-63, g odd: 64-127) so consecutive groups'
LDWEIGHTS can pull ahead and their matmuls run concurrently.
Measured on HW (test.py loop-slope): 150278 ns (fp32r baseline)
-> 134466 (bf16 operands, merged exp, band alternation)
-> 132468 (fused prep, bf16 x/weights)
-> 128378 (For_i staggered_reset + PE branch-prefetch hints;
   cold-process spread 128-137, warm in-process 122-127).
kernel_v7.py holds two further unproven candidates (phase-1 prep
evacuations double-buffered through one shared 2-slot PSUM pool;
bf16 output DMA) -- correctness-verified but never cleanly timed:
the device session degraded (erratic 135-165us, negative slopes)
before a trustworthy A/B could be taken.
PE-busy floor model: PV 512x(107.5 stream + 53 LDW + 4 NX) = 84us,
energy ~14.5us (2-band concurrent), prep ~17us -> ~119us; the wall
is PE-bound, ACT exp is ~77us busy and off the critical path.
"""

import numpy as np

E_PACK = True  # row-tile the K=32 energy matmuls via tile_position
HEAD_SPLIT = True  # 256-col first chunk: first matmul starts earlier
PREP_FUSE = True  # weave chunk-0 attention into the prep phase
OUT_ONE_DMA = True  # one output DMA per n-chunk instead of per n-sub
E_BF16 = True  # Q'/K' in bf16: FWL fast weight loads on energy matmuls
VT_STT = True  # V bias via one DVE scalar_tensor_tensor instead of K=1 matmuls
EXP_SPLIT = False  # two ACT instructions per group: PV can start after the first half
ABL_EXP_HALF = False  # timing ablation: exp over half the tile (WRONG results)
ABL_PV_HALF = False  # timing ablation: skip half the PV matmuls (WRONG results)
MG = 2  # m-tiles per exp group (<=4; MG concurrent PE row-bands). MG=4 with
# NCHW=256 is ILLEGAL: the band matmuls would write 256-col halves of the
# same PSUM bank concurrently -> fatal bank write collision on HW.
NCHW = 512  # n-chunk width (columns of Q processed per oT accumulation pass)
E_BUFS = 2  # psum double/triple buffering for energy tiles
PV_BF16 = True  # store P and V_aug as bf16: FWL fast weight load on PV matmuls
E_BAND_ALT = True  # alternate PE row-bands across energy groups: LDW overlaps
SPLIT_LDW = False  # bacc already auto-splits LDW from matmuls; extras load twice
X_BF16 = True  # xf/wpack in bf16: FWL on V-prep/KQ-proj stationaries, half DMA
LOOP_HINTS = True  # For_i branch prefetch hints (body >256 PE instructions)
OUT_BF16 = False  # bf16 output DMA: halves out traffic, shorter tail

B = 4
C = 256
D = 32
N = 4096
NQ = N // 2  # query columns per core
N_CORES = 8

_CACHE = {}


def _patch_tile_drain(tile_mod, mybir):
    """Walrus in this container rejects >2 sync waits on a Drain
    instruction ("Too many sync wait commands"). Split the TileContext
    tail-drain waits into single-wait NOPs instead."""
    if getattr(tile_mod.TileContext, "_drain_patched", False):
        return

    def _drain_and_barrier(self, tick_clock, wait_clock):
        from concourse.tile import ScopedClock

        drain_inst = self.nc.sync.drain()
        wait_clock.add_sem_waits(
            drain_inst.ins, ScopedClock({None: tick_clock.global_clock})
        )
        si = drain_inst.ins.sync_info
        if si is not None and len(si.on_wait) > 1:
            waits = list(si.on_wait)
            # Engine-owned sems are re-synchronized by the all_engine_barrier
            # below; only DMA-queue sems (no engine) must be awaited here.
            dma_waits = [w for w in waits if "DMA" in (w.ant_name or "")]
            eng_waits = [w for w in waits if "DMA" not in (w.ant_name or "")]
            keep = dma_waits if dma_waits else waits
            drain_inst.ins.sync_info = mybir.SyncInfo(
                on_wait=[keep[0]], on_update=[]
            )
            for w in keep[1:]:
                n = self.nc.sync.nop()
                n.ins.sync_info = mybir.SyncInfo(on_wait=[w], on_update=[])
        self.nc.all_engine_barrier()
        popped = self.nc._tile_sem_poison_stack.pop()
        assert popped is self._sem_poison
        self.nc.clear_and_free_semaphores(list(self.sems.allocated().values()))
        self.nc.all_engine_barrier()

    tile_mod.TileContext._drain_and_barrier = _drain_and_barrier
    tile_mod.TileContext._drain_patched = True


def _split_sync_waits(nc, mybir, max_waits=1):
    """This container's walrus rejects instructions carrying more than
    ~1 sync wait (e.g. Matmult/S3_LW, Drain/CTRL). Hoist extra waits onto
    same-engine NOPs placed immediately before the instruction."""
    ctr = [0]
    for fn in nc.m.functions:
        for blk in fn.blocks:
            new_insts = []
            for inst in blk.instructions:
                si = inst.sync_info
                if si is not None and len(si.on_wait) > max_waits:
                    waits = list(si.on_wait)
                    for w in waits[max_waits:]:
                        ctr[0] += 1
                        nop = mybir.InstNoOp(
                            name=f"WSPLIT-{ctr[0]}",
                            sync_info=mybir.SyncInfo(on_wait=[w], on_update=[]),
                            bass_nofuse=True,
                            engine=inst.engine,
                        )
                        new_insts.append(nop)
                    inst.sync_info = mybir.SyncInfo(
                        on_wait=waits[:max_waits], on_update=list(si.on_update)
                    )
                new_insts.append(inst)
            blk.instructions = new_insts


def _build_graph(repeat=1, loop_n=1):
    import sys

    if "/opt/trn_rl_repo" not in sys.path:
        sys.path.insert(0, "/opt/trn_rl_repo")
    import concourse.bass as bass
    import concourse.mybir as mybir
    import concourse.tile as tile

    _patch_tile_drain(tile, mybir)

    F32 = mybir.dt.float32
    F32R = mybir.dt.float32r
    BF16 = mybir.dt.bfloat16
    PVDT = BF16 if PV_BF16 else F32R
    EDT = BF16 if E_BF16 in (True, 1) else F32R
    KDT = BF16 if E_BF16 in (True, 1, 2) else F32R
    XDT = BF16 if X_BF16 else F32R
    Exp = mybir.ActivationFunctionType.Exp
    MULT = mybir.AluOpType.mult
    ADD = mybir.AluOpType.add

    _split_pairs = []  # names of InstMatmult with ldweights=False

    nc = bass.Bass()
    xf = nc.declare_dram_parameter("xf", [C, N], XDT, isOutput=False)
    xqT = nc.declare_dram_parameter("xqT", [NQ, C], F32, isOutput=False)
    wpack = nc.declare_dram_parameter("wpack", [C, 512], XDT, isOutput=False)
    bpack = nc.declare_dram_parameter("bpack", [128, 2], F32, isOutput=False)
    ODT = BF16 if OUT_BF16 else F32
    outp = nc.declare_dram_parameter("out", [NQ, C], ODT, isOutput=True)

    MT = N // 128  # 32 m-tiles
    VW = C + 2  # 258: V_T plus ones column plus pad (fp32r needs even free dim)

    with tile.TileContext(nc) as tc:
        with tc.tile_pool(name="consts", bufs=1) as consts:
            # packed weights/biases first (few DMAs) so projections start fast
            # wp_s layout per c-half h (rows h*128..): [w1t | w2t | w3t] cols
            wp_s = consts.tile([128, 1024], XDT)
            nc.sync.dma_start(wp_s[:, 0:512], wpack[0:128, :])
            nc.sync.dma_start(wp_s[:, 512:1024], wpack[128:256, :])
            bp_s = consts.tile([128, 2], F32)
            w1t_s = [wp_s[:, 0:128], wp_s[:, 512:640]]  # per c-half
            w2t_s = [wp_s[:, 128:256], wp_s[:, 640:768]]
            w3t_s = [wp_s[:, 256:512], wp_s[:, 768:1024]]
            b1r_s = bp_s[:, 0:1]
            b2r_s = bp_s[:, 1:2]
            eshift_s = consts.tile([128, 1], F32)
            nc.vector.memset(eshift_s[:], -25.0)
            # xf in interleaved half-pairs so chunk-ch projections unblock
            # as soon as their slices land
            xf_s = consts.tile([128, 2 * N], XDT)
            if HEAD_SPLIT:
                for q in range(2):  # chunk 0 in 256-col quarters
                    nc.sync.dma_start(
                        xf_s[:, q * 256 : (q + 1) * 256],
                        xf[0:128, q * 256 : (q + 1) * 256],
                    )
                    nc.sync.dma_start(
                        xf_s[:, N + q * 256 : N + (q + 1) * 256],
                        xf[128:256, q * 256 : (q + 1) * 256],
                    )
                    if q == 0:
                        nc.sync.dma_start(bp_s[:], bpack[:])
            else:
                nc.sync.dma_start(bp_s[:], bpack[:])
            for ch in range(0 if not HEAD_SPLIT else 1, 8):
                nc.sync.dma_start(
                    xf_s[:, ch * 512 : (ch + 1) * 512],
                    xf[0:128, ch * 512 : (ch + 1) * 512],
                )
                nc.sync.dma_start(
                    xf_s[:, N + ch * 512 : N + (ch + 1) * 512],
                    xf[128:256, ch * 512 : (ch + 1) * 512],
                )
            xqT_s = consts.tile([128, (NQ // 128) * C], F32)
            for s in range(NQ // 128):
                nc.sync.dma_start(
                    xqT_s[:, s * C : (s + 1) * C], xqT[s * 128 : (s + 1) * 128, :]
                )

            qp_s = consts.tile([128, NQ], EDT)  # Q' replicated x4 on partitions
            kp_s = consts.tile([128, N], KDT)  # K' replicated x4
            vt_s = consts.tile([128, MT * VW], PVDT)  # [V_T | 1 | 0] per m-tile

            import contextlib

            loop_kw = (
                dict(
                    hint_engines=(mybir.EngineType.PE,),
                    staggered_reset=True,
                )
                if LOOP_HINTS
                else {}
            )
            loop_ctx = (
                tc.For_i(0, loop_n, 1, **loop_kw)
                if loop_n > 1
                else contextlib.nullcontext()
            )
            with loop_ctx:
              for _rep in range(repeat):
                # ---- compute body ----
                # Prep (K'/Q'/V_T projections) is emitted per 512-col chunk so
                # it tracks the xf DMA stream. With PREP_FUSE, chunk-0 of the
                # attention is woven between prep chunks (its group g needs
                # only the m-tiles prep chunk g//2 just produced), and the
                # prep PSUM tiles borrow the energy pool's slots so total
                # PSUM stays within 8 banks.
                NSUB = NCHW // 128
                NCH = NQ // NCHW
                NG = MT // MG
                import contextlib as _ctb

                with (
                    tc.tile_pool(name="ppool", bufs=4) as ppool,
                    tc.tile_pool(name="opool", bufs=4) as opool,
                    tc.tile_pool(name="zpool", bufs=6) as zpool,
                    _ctb.ExitStack() as _psum_stack,
                ):
                    _pools = {}

                    def open_po():
                        # 2 chunks of oT accumulators in flight (when they
                        # fit in PSUM next to the e tiles): the next chunk's
                        # PV matmuls start while the previous chunk's
                        # finalize is still reading its oT tiles.
                        po_bufs = 2 * NSUB if NSUB <= 2 else NSUB
                        _pools["po"] = _psum_stack.enter_context(
                            tc.tile_pool(name="po", bufs=po_bufs, space="PSUM")
                        )

                    def open_pe():
                        _pools["pe"] = _psum_stack.enter_context(
                            tc.tile_pool(name="pe", bufs=E_BUFS, space="PSUM")
                        )

                    def open_psum_pools():
                        open_pe()
                        open_po()

                    if _rep == 0:
                        vt_view = vt_s.rearrange("p (t w) -> p t w", w=VW)
                        mset_dt = BF16 if PV_BF16 else F32
                        nc.vector.memset(vt_view[:, :, C].bitcast(mset_dt), 1.0)
                        nc.vector.memset(vt_view[:, :, C + 1].bitcast(mset_dt), 0.0)

                    def _emit_v_tile(t, pool_v, v_tag):
                        ps = pool_v.tile(
                            [128, C], F32, name=f"r{_rep}psv{t}", tag=v_tag
                        )
                        nc.tensor.matmul(
                            ps[:],
                            xf_s[:, t * 128 : (t + 1) * 128],
                            w3t_s[0],
                            start=True,
                            stop=False,
                        )
                        nc.tensor.matmul(
                            ps[:],
                            xf_s[:, N + t * 128 : N + (t + 1) * 128],
                            w3t_s[1],
                            start=False,
                            stop=True,
                        )
                        nc.vector.tensor_copy(vt_s[:, t * VW : t * VW + C], ps[:])

                    def emit_proj_piece(pool, tag, w_s, b_s, dst, lo, w):
                        ps = pool.tile(
                            [128, w], F32, name=f"r{_rep}pp{lo}_{w}", tag=tag
                        )
                        nc.tensor.matmul(
                            ps[:], w_s[0], xf_s[:, lo : lo + w], start=True, stop=False
                        )
                        nc.tensor.matmul(
                            ps[:],
                            w_s[1],
                            xf_s[:, N + lo : N + lo + w],
                            start=False,
                            stop=True,
                        )
                        nc.vector.tensor_scalar_add(dst[:, lo : lo + w], ps[:], b_s)

                    def emit_K(ch, pool, tag):
                        emit_proj_piece(pool, tag, w2t_s, b2r_s, kp_s, ch * 512, 512)

                    def emit_Q(ch, pool, tag):
                        emit_proj_piece(pool, tag, w1t_s, b1r_s, qp_s, ch * 512, 512)

                    def emit_prep_chunk(ch, pool_kq, pool_v=None):
                        if pool_v is None:
                            pool_v = pool_kq
                        kq_tag = "e" if pool_kq is _pools.get("pe") else "psk"
                        v_tag = "e" if pool_v is _pools.get("pe") else "psv"
                        if ch == 0 and HEAD_SPLIT:
                            for q in range(2):
                                emit_proj_piece(
                                    pool_kq, kq_tag, w2t_s, b2r_s, kp_s, q * 256, 256
                                )
                            for q in range(2):
                                emit_proj_piece(
                                    pool_kq, kq_tag, w1t_s, b1r_s, qp_s, q * 256, 256
                                )
                            for t in range(4):
                                _emit_v_tile(t, pool_v, v_tag)
                            return
                        ps = pool_kq.tile(
                            [128, 512], F32, name=f"r{_rep}psk{ch}", tag=kq_tag
                        )
                        nc.tensor.matmul(
                            ps[:],
                            w2t_s[0],
                            xf_s[:, ch * 512 : (ch + 1) * 512],
                            start=True,
                            stop=False,
                        )
                        nc.tensor.matmul(
                            ps[:],
                            w2t_s[1],
                            xf_s[:, N + ch * 512 : N + (ch + 1) * 512],
                            start=False,
                            stop=True,
                        )
                        nc.vector.tensor_scalar_add(
                            kp_s[:, ch * 512 : (ch + 1) * 512], ps[:], b2r_s
                        )
                        if ch < NQ // 512:
                            ps = pool_kq.tile(
                                [128, 512], F32, name=f"r{_rep}psq{ch}", tag=kq_tag
                            )
                            nc.tensor.matmul(
                                ps[:],
                                w1t_s[0],
                                xf_s[:, ch * 512 : (ch + 1) * 512],
                                start=True,
                                stop=False,
                            )
                            nc.tensor.matmul(
                                ps[:],
                                w1t_s[1],
                                xf_s[:, N + ch * 512 : N + (ch + 1) * 512],
                                start=False,
                                stop=True,
                            )
                            nc.vector.tensor_scalar_add(
                                qp_s[:, ch * 512 : (ch + 1) * 512], ps[:], b1r_s
                            )
                        for t in range(ch * 4, ch * 4 + 4):
                            _emit_v_tile(t, pool_v, v_tag)

                    def emit_E(c, g, pool=None):
                        e = (pool or _pools["pe"]).tile(
                            [128, MG * NCHW], F32, name=f"r{_rep}e{c}_{g}", tag="e"
                        )
                        nbands = 4 // MG if E_BAND_ALT else 1
                        for i in range(MG):
                            mt = MG * g + i
                            band = (
                                32 * (MG * (g % nbands) + i) if E_PACK else 0
                            )
                            kw = dict(tile_position=(band, 0)) if E_PACK else {}
                            w_ap = kp_s[
                                band : band + 32, mt * 128 : (mt + 1) * 128
                            ]
                            if SPLIT_LDW and KDT is BF16:
                                nc.tensor.ldweights(w_ap, **kw)
                            mm = nc.tensor.matmul(
                                e[:, i * NCHW : (i + 1) * NCHW],
                                w_ap,
                                qp_s[band : band + 32, c * NCHW : (c + 1) * NCHW],
                                start=True,
                                stop=True,
                                **kw,
                            )
                            if SPLIT_LDW and KDT is BF16:
                                mm.ins.ldweights = False
                                _split_pairs.append(mm.ins.name)
                        return e

                    def make_oT(c):
                        return [
                            _pools["po"].tile(
                                [128, VW], F32, name=f"r{_rep}oT{c}_{s}", tag="oT"
                            )
                            for s in range(NSUB)
                        ]

                    def emit_group(c, g, e_cur, oT):
                        # exp(E - ESHIFT): softmax is shift-invariant; the
                        # constant rides the ACT instruction's free affine and
                        # moves the fp32 overflow ceiling from |E|~88 to ~113.
                        p = ppool.tile(
                            [128, MG * NCHW], PVDT, name=f"r{_rep}p{c}_{g}", tag="p"
                        )
                        if EXP_SPLIT:
                            h = MG * NCHW // 2
                            nc.scalar.activation(
                                p[:, 0:h], e_cur[:, 0:h], Exp, bias=eshift_s[:]
                            )
                            nc.scalar.activation(
                                p[:, h:], e_cur[:, h:], Exp, bias=eshift_s[:]
                            )
                        else:
                            nc.scalar.activation(
                                p[:], e_cur[:], Exp, bias=eshift_s[:]
                            )
                        for i in range(MG):
                            mt = MG * g + i
                            for s in range(NSUB):
                                nc.tensor.matmul(
                                    oT[s][:],
                                    p[:, i * NCHW + s * 128 : i * NCHW + (s + 1) * 128],
                                    vt_s[:, mt * VW : (mt + 1) * VW],
                                    start=(g == 0 and i == 0),
                                    stop=(g == NG - 1 and i == MG - 1),
                                )

                    def emit_finalize(c, oT, per_sub_dma=False):
                        otc = opool.tile(
                            [128, NSUB * C], ODT, name=f"r{_rep}otc{c}", tag="ot"
                        )
                        for s in range(NSUB):
                            ns = c * NSUB + s
                            zinv = zpool.tile(
                                [128, 1], F32, name=f"r{_rep}zinv{c}_{s}", tag="zinv"
                            )
                            nc.vector.reciprocal(zinv[:], oT[s][:, C : C + 1])
                            nc.vector.scalar_tensor_tensor(
                                otc[:, s * C : (s + 1) * C],
                                oT[s][:, 0:C],
                                zinv[:],
                                xqT_s[:, ns * C : (ns + 1) * C],
                                MULT,
                                ADD,
                            )
                            if per_sub_dma:
                                nc.sync.dma_start(
                                    outp[ns * 128 : (ns + 1) * 128, :],
                                    otc[:, s * C : (s + 1) * C],
                                )
                        if not per_sub_dma:
                            dst = outp[
                                c * NSUB * 128 : (c + 1) * NSUB * 128, :
                            ].rearrange("(s p) c -> p s c", p=128)
                            nc.sync.dma_start(
                                dst, otc.rearrange("p (s c) -> p s c", c=C)
                            )

                    if PREP_FUSE:
                        open_po()
                        oT = make_oT(0)
                        with (
                            tc.tile_pool(name="pp", bufs=1, space="PSUM") as pp,
                            tc.tile_pool(name="pv", bufs=1, space="PSUM") as pv,
                            tc.tile_pool(name="pe1", bufs=1, space="PSUM") as pe1,
                        ):
                            gpc = 4 // MG  # chunk-0 groups per prep chunk
                            for ch in range(N // 512):
                                emit_K(ch, pp, "psk")
                                if ch == 0:
                                    emit_Q(0, pp, "psk")
                                _emit_v_tile(4 * ch + 0, pv, "psv")
                                _emit_v_tile(4 * ch + 1, pv, "psv")
                                if gpc == 2:
                                    g = 2 * ch
                                    emit_group(0, g, emit_E(0, g, pe1), oT)
                                _emit_v_tile(4 * ch + 2, pv, "psv")
                                _emit_v_tile(4 * ch + 3, pv, "psv")
                                if gpc == 2:
                                    g = 2 * ch + 1
                                else:
                                    g = ch
                                emit_group(0, g, emit_E(0, g, pe1), oT)
                                if 0 < ch < NQ // 512:
                                    emit_Q(ch, pp, "psk")
                        emit_finalize(0, oT)
                        open_pe()
                        # phase 2: remaining chunks, software-pipelined
                        steps = [(c, g) for c in range(1, NCH) for g in range(NG)]
                        e_cur = emit_E(*steps[0])
                        for k, (c, g) in enumerate(steps):
                            if g == 0:
                                oT = make_oT(c)
                            e_next = (
                                emit_E(*steps[k + 1]) if k + 1 < len(steps) else None
                            )
                            emit_group(c, g, e_cur, oT)
                            e_cur = e_next
                            if g == NG - 1:
                                emit_finalize(c, oT, per_sub_dma=(c == NCH - 1))
                    else:
                        with (
                            tc.tile_pool(name="pp", bufs=2, space="PSUM") as pp,
                            tc.tile_pool(name="pv", bufs=2, space="PSUM") as pv,
                        ):
                            for ch in range(N // 512):
                                emit_prep_chunk(ch, pp, pv)
                        open_psum_pools()
                        steps = [(c, g) for c in range(NCH) for g in range(NG)]
                        e_cur = emit_E(*steps[0])
                        for k, (c, g) in enumerate(steps):
                            if g == 0:
                                oT = make_oT(c)
                            e_next = (
                                emit_E(*steps[k + 1]) if k + 1 < len(steps) else None
                            )
                            emit_group(c, g, e_cur, oT)
                            e_cur = e_next
                            if g == NG - 1:
                                emit_finalize(c, oT, per_sub_dma=(c == NCH - 1))

    _verify_split_ldw(nc, mybir, set(_split_pairs))
    _split_sync_waits(nc, mybir)
    return nc


def _verify_split_ldw(nc, mybir, pair_names):
    """A matmul with ldweights=False computes with whatever the PE array
    holds; ensure the immediately-preceding weight-touching PE instruction
    is its own InstLdweights (same weights AP). If the Tile scheduler
    moved anything in between, restore the self-loading mode."""
    if not pair_names:
        return
    refused = 0
    for fn in nc.m.functions:
        for blk in fn.blocks:
            last_w = None  # (kind, weights_ap_repr)
            for inst in blk.instructions:
                if isinstance(inst, mybir.InstLdweights):
                    last_w = ("ldw", repr(inst.ins[0]))
                elif isinstance(inst, mybir.InstMatmult):
                    if inst.name in pair_names:
                        ok = (
                            last_w is not None
                            and last_w[0] == "ldw"
                            and last_w[1] == repr(inst.ins[1])
                        )
                        if not ok:
                            inst.ldweights = True
                            refused += 1
                    last_w = ("mm", None)
    if refused:
        import logging

        logging.getLogger(__name__).warning(
            f"_verify_split_ldw: re-fused {refused} matmuls"
        )


def _get_graph(repeat=1, loop_n=1):
    key = f"nc{repeat}_{loop_n}"
    if key not in _CACHE:
        _CACHE[key] = _build_graph(repeat, loop_n)
    return _CACHE[key]


def _host_inputs(x, w1, b1, w2, b2, w3, b3):
    x = np.asarray(x, dtype=np.float32)
    xf = x.reshape(B, C, N)
    w1t = np.tile(np.asarray(w1, np.float32).T, (1, 4))  # [256, 128]
    w2t = np.tile(np.asarray(w2, np.float32).T, (1, 4))
    w3t = np.ascontiguousarray(np.asarray(w3, np.float32).T)  # [256, 256]
    wpack = np.concatenate([w1t, w2t, w3t], axis=1)  # [256, 512]
    if X_BF16:
        import ml_dtypes

        wpack = wpack.astype(ml_dtypes.bfloat16)
    bpack = np.stack(
        [np.tile(np.asarray(b1, np.float32), 4), np.tile(np.asarray(b2, np.float32), 4)],
        axis=1,
    )  # [128, 2]
    b3f = np.asarray(b3, np.float32)

    in_maps = []
    for core in range(N_CORES):
        b, half = divmod(core, 2)
        n0 = half * NQ
        xb = xf[b]
        # roll so this core's query range sits at columns 0:NQ; K/V sums
        # over m are permutation-invariant so the roll is harmless.
        x_roll = np.ascontiguousarray(np.roll(xb, -n0, axis=1))
        if X_BF16:
            import ml_dtypes

            x_roll = x_roll.astype(ml_dtypes.bfloat16)
        xqTc = np.ascontiguousarray(xb[:, n0 : n0 + NQ].T + b3f[None, :])
        in_maps.append(
            {"xf": x_roll, "xqT": xqTc, "wpack": wpack, "bpack": bpack}
        )
    return in_maps


def kernel(x, w1, b1, w2, b2, w3, b3, _trace=False, _repeat=1):
    import sys

    if "/opt/trn_rl_repo" not in sys.path:
        sys.path.insert(0, "/opt/trn_rl_repo")
    from concourse.bass_utils import run_bass_kernel_spmd

    nc = _get_graph(_repeat)
    in_maps = _host_inputs(x, w1, b1, w2, b2, w3, b3)
    res = run_bass_kernel_spmd(nc, in_maps, list(range(N_CORES)), trace=_trace)
    _CACHE["last_result"] = res

    out = np.empty((B, C, N), np.float32)
    for core in range(N_CORES):
        b, half = divmod(core, 2)
        n0 = half * NQ
        out[b][:, n0 : n0 + NQ] = res.results[core]["out"].astype(np.float32).T
    return out.reshape(B, C, 64, 64)



# revision 34
# speedup vs baseline: 1.2151x; 1.0719x over previous
"""
Self-attention (SAGAN-style) Bass kernel for Trainium2, 8 NeuronCores.

Reference computation (per batch b, X = x[b] reshaped [C=256, N=4096]):
    q = W1 X + b1          [D=32, N]
    k = W2 X + b2          [D=32, N]
    v = W3 X + b3          [C, N]
    E[n, m]   = q[:, n] . k[:, m]
    A         = softmax over m of E
    out[c, n] = sum_m v[c, m] A[n, m]  + X[c, n]

Sharding: 8 cores = 4 batches x 2 query-halves. Each core gets the full
X for its batch (needed for K/V over all N) but computes only its half of
the query rows -> no inter-core communication at all.

Device algorithm (per core, NQ = 2048 query columns):
  - Q' = W1 X[:, 0:NQ], K' = W2 X, each replicated 4x along partitions
    (host pre-tiles the weights) so the K=32 energy matmuls can use
    PE row-tiling.
  - V_T[m, c] = (X^T W3^T)[m, c] computed natively (no transposes), with
    an appended constant ones column: V_aug = [V_T | 1 | 0pad]. The V bias
    b3 drops out of the matmuls entirely: its contribution to out_T is
    Z[n]*b3[c], which after the /Z normalization is just +b3 -- folded
    into the host-prepared residual xqT.
  - E^T[m, n] = K'^T Q' with m on partitions -> exp(E - 25) without any
    per-row max subtraction (softmax is shift-invariant; the constant
    rides the ACT affine and keeps fp32 safe up to |E| ~ 113) -> P^T. The two K=32 energy
    matmuls of each group run concurrently in separate PE row-bands
    (tile_position), and exp is split in two so PV can start after the
    first half.
  - out_T[n, c], Z[n] = P^T.T @ V_aug accumulated over all m in PSUM:
    the ones column yields the softmax denominator for free.
  - out = out_T[:, 0:256] * (1/Z) + (X_q^T + b3)  (one fused vector op),
    DMA out in [n, c] layout; the host transposes back when unsharding.

All matmul operands are stored as bf16 (x, weights, Q', K', P, V_aug):
the PE streams 1 column/cycle either way, but bf16 halves the
LDWEIGHTS cost via FWL (fast weight load) — and on this toolchain the
per-matmul weight load is fully serialized with the matmul (measured
156 ns for LDW+MM at 258 free-dim), so LDW time is on the critical
path everywhere. PSUM accumulation stays fp32; the residual/normalize
path (xqT, biases, 1/Z) stays fp32. Energy groups alternate PE row
bands (g even: rows 0-63, g odd: 64-127) so consecutive groups'
LDWEIGHTS can pull ahead and their matmuls run concurrently.
Measured on HW (test.py loop-slope): 150278 ns (fp32r baseline)
-> 134466 (bf16 operands, merged exp, band alternation)
-> 132468 (fused prep, bf16 x/weights)
-> 128378 (For_i staggered_reset + PE branch-prefetch hints;
   cold-process spread 128-137, warm in-process 122-127)
-> 120642 (same kernel, spike-robust measurement in test.py: the axon
   RPC adds random ~+39ms spikes to ~40% of calls which corrupted the
   median slope; the guarded estimator recovers the clean value).
kernel_v7.py (phase-1 prep evacuations through one shared 2-slot
PSUM pool + bf16 output DMA) was REJECTED: bracketed paired-median
A/B measured it ~8us slower -- cycling all 7-8 prep allocations per
chunk through 2 shared slots serializes phase-1 worse than the
split single-buffered pp/pv pools.
PE-busy floor model: PV 512x(107.5 stream + 53 LDW + 4 NX) = 84us,
energy ~14.5us (2-band concurrent), prep ~17us -> ~119us; the wall
is PE-bound, ACT exp is ~77us busy and off the critical path.
"""

import numpy as np

E_PACK = True  # row-tile the K=32 energy matmuls via tile_position
HEAD_SPLIT = True  # 256-col first chunk: first matmul starts earlier
PREP_FUSE = True  # weave chunk-0 attention into the prep phase
OUT_ONE_DMA = True  # one output DMA per n-chunk instead of per n-sub
E_BF16 = True  # Q'/K' in bf16: FWL fast weight loads on energy matmuls
VT_STT = True  # V bias via one DVE scalar_tensor_tensor instead of K=1 matmuls
EXP_SPLIT = False  # two ACT instructions per group: PV can start after the first half
ABL_EXP_HALF = False  # timing ablation: exp over half the tile (WRONG results)
ABL_PV_HALF = False  # timing ablation: skip half the PV matmuls (WRONG results)
MG = 2  # m-tiles per exp group (<=4; MG concurrent PE row-bands). MG=4 with
# NCHW=256 is ILLEGAL: the band matmuls would write 256-col halves of the
# same PSUM bank concurrently -> fatal bank write collision on HW.
NCHW = 512  # n-chunk width (columns of Q processed per oT accumulation pass)
E_BUFS = 2  # psum double/triple buffering for energy tiles
PV_BF16 = True  # store P and V_aug as bf16: FWL fast weight load on PV matmuls
E_BAND_ALT = True  # alternate PE row-bands across energy groups: LDW overlaps
SPLIT_LDW = False  # bacc already auto-splits LDW from matmuls; extras load twice
X_BF16 = True  # xf/wpack in bf16: FWL on V-prep/KQ-proj stationaries, half DMA
LOOP_HINTS = True  # For_i branch prefetch hints (body >256 PE instructions)
OUT_BF16 = False  # bf16 output DMA: halves out traffic, shorter tail

B = 4
C = 256
D = 32
N = 4096
NQ = N // 2  # query columns per core
N_CORES = 8

_CACHE = {}


def _patch_tile_drain(tile_mod, mybir):
    """Walrus in this container rejects >2 sync waits on a Drain
    instruction ("Too many sync wait commands"). Split the TileContext
    tail-drain waits into single-wait NOPs instead."""
    if getattr(tile_mod.TileContext, "_drain_patched", False):
        return

    def _drain_and_barrier(self, tick_clock, wait_clock):
        from concourse.tile import ScopedClock

        drain_inst = self.nc.sync.drain()
        wait_clock.add_sem_waits(
            drain_inst.ins, ScopedClock({None: tick_clock.global_clock})
        )
        si = drain_inst.ins.sync_info
        if si is not None and len(si.on_wait) > 1:
            waits = list(si.on_wait)
            # Engine-owned sems are re-synchronized by the all_engine_barrier
            # below; only DMA-queue sems (no engine) must be awaited here.
            dma_waits = [w for w in waits if "DMA" in (w.ant_name or "")]
            eng_waits = [w for w in waits if "DMA" not in (w.ant_name or "")]
            keep = dma_waits if dma_waits else waits
            drain_inst.ins.sync_info = mybir.SyncInfo(
                on_wait=[keep[0]], on_update=[]
            )
            for w in keep[1:]:
                n = self.nc.sync.nop()
                n.ins.sync_info = mybir.SyncInfo(on_wait=[w], on_update=[])
        self.nc.all_engine_barrier()
        popped = self.nc._tile_sem_poison_stack.pop()
        assert popped is self._sem_poison
        self.nc.clear_and_free_semaphores(list(self.sems.allocated().values()))
        self.nc.all_engine_barrier()

    tile_mod.TileContext._drain_and_barrier = _drain_and_barrier
    tile_mod.TileContext._drain_patched = True


def _split_sync_waits(nc, mybir, max_waits=1):
    """This container's walrus rejects instructions carrying more than
    ~1 sync wait (e.g. Matmult/S3_LW, Drain/CTRL). Hoist extra waits onto
    same-engine NOPs placed immediately before the instruction."""
    ctr = [0]
    for fn in nc.m.functions:
        for blk in fn.blocks:
            new_insts = []
            for inst in blk.instructions:
                si = inst.sync_info
                if si is not None and len(si.on_wait) > max_waits:
                    waits = list(si.on_wait)
                    for w in waits[max_waits:]:
                        ctr[0] += 1
                        nop = mybir.InstNoOp(
                            name=f"WSPLIT-{ctr[0]}",
                            sync_info=mybir.SyncInfo(on_wait=[w], on_update=[]),
                            bass_nofuse=True,
                            engine=inst.engine,
                        )
                        new_insts.append(nop)
                    inst.sync_info = mybir.SyncInfo(
                        on_wait=waits[:max_waits], on_update=list(si.on_update)
                    )
                new_insts.append(inst)
            blk.instructions = new_insts


def _build_graph(repeat=1, loop_n=1):
    import sys

    if "/opt/trn_rl_repo" not in sys.path:
        sys.path.insert(0, "/opt/trn_rl_repo")
    import concourse.bass as bass
    import concourse.mybir as mybir
    import concourse.tile as tile

    _patch_tile_drain(tile, mybir)

    F32 = mybir.dt.float32
    F32R = mybir.dt.float32r
    BF16 = mybir.dt.bfloat16
    PVDT = BF16 if PV_BF16 else F32R
    EDT = BF16 if E_BF16 in (True, 1) else F32R
    KDT = BF16 if E_BF16 in (True, 1, 2) else F32R
    XDT = BF16 if X_BF16 else F32R
    Exp = mybir.ActivationFunctionType.Exp
    MULT = mybir.AluOpType.mult
    ADD = mybir.AluOpType.add

    _split_pairs = []  # names of InstMatmult with ldweights=False

    nc = bass.Bass()
    xf = nc.declare_dram_parameter("xf", [C, N], XDT, isOutput=False)
    xqT = nc.declare_dram_parameter("xqT", [NQ, C], F32, isOutput=False)
    wpack = nc.declare_dram_parameter("wpack", [C, 512], XDT, isOutput=False)
    bpack = nc.declare_dram_parameter("bpack", [128, 2], F32, isOutput=False)
    ODT = BF16 if OUT_BF16 else F32
    outp = nc.declare_dram_parameter("out", [NQ, C], ODT, isOutput=True)

    MT = N // 128  # 32 m-tiles
    VW = C + 2  # 258: V_T plus ones column plus pad (fp32r needs even free dim)

    with tile.TileContext(nc) as tc:
        with tc.tile_pool(name="consts", bufs=1) as consts:
            # packed weights/biases first (few DMAs) so projections start fast
            # wp_s layout per c-half h (rows h*128..): [w1t | w2t | w3t] cols
            wp_s = consts.tile([128, 1024], XDT)
            nc.sync.dma_start(wp_s[:, 0:512], wpack[0:128, :])
            nc.sync.dma_start(wp_s[:, 512:1024], wpack[128:256, :])
            bp_s = consts.tile([128, 2], F32)
            w1t_s = [wp_s[:, 0:128], wp_s[:, 512:640]]  # per c-half
            w2t_s = [wp_s[:, 128:256], wp_s[:, 640:768]]
            w3t_s = [wp_s[:, 256:512], wp_s[:, 768:1024]]
            b1r_s = bp_s[:, 0:1]
            b2r_s = bp_s[:, 1:2]
            eshift_s = consts.tile([128, 1], F32)
            nc.vector.memset(eshift_s[:], -25.0)
            # xf in interleaved half-pairs so chunk-ch projections unblock
            # as soon as their slices land
            xf_s = consts.tile([128, 2 * N], XDT)
            if HEAD_SPLIT:
                for q in range(2):  # chunk 0 in 256-col quarters
                    nc.sync.dma_start(
                        xf_s[:, q * 256 : (q + 1) * 256],
                        xf[0:128, q * 256 : (q + 1) * 256],
                    )
                    nc.sync.dma_start(
                        xf_s[:, N + q * 256 : N + (q + 1) * 256],
                        xf[128:256, q * 256 : (q + 1) * 256],
                    )
                    if q == 0:
                        nc.sync.dma_start(bp_s[:], bpack[:])
            else:
                nc.sync.dma_start(bp_s[:], bpack[:])
            for ch in range(0 if not HEAD_SPLIT else 1, 8):
                nc.sync.dma_start(
                    xf_s[:, ch * 512 : (ch + 1) * 512],
                    xf[0:128, ch * 512 : (ch + 1) * 512],
                )
                nc.sync.dma_start(
                    xf_s[:, N + ch * 512 : N + (ch + 1) * 512],
                    xf[128:256, ch * 512 : (ch + 1) * 512],
                )
            xqT_s = consts.tile([128, (NQ // 128) * C], F32)
            for s in range(NQ // 128):
                nc.sync.dma_start(
                    xqT_s[:, s * C : (s + 1) * C], xqT[s * 128 : (s + 1) * 128, :]
                )

            qp_s = consts.tile([128, NQ], EDT)  # Q' replicated x4 on partitions
            kp_s = consts.tile([128, N], KDT)  # K' replicated x4
            vt_s = consts.tile([128, MT * VW], PVDT)  # [V_T | 1 | 0] per m-tile

            import contextlib

            loop_kw = (
                dict(
                    hint_engines=(mybir.EngineType.PE,),
                    staggered_reset=True,
                )
                if LOOP_HINTS
                else {}
            )
            loop_ctx = (
                tc.For_i(0, loop_n, 1, **loop_kw)
                if loop_n > 1
                else contextlib.nullcontext()
            )
            with loop_ctx:
              for _rep in range(repeat):
                # ---- compute body ----
                # Prep (K'/Q'/V_T projections) is emitted per 512-col chunk so
                # it tracks the xf DMA stream. With PREP_FUSE, chunk-0 of the
                # attention is woven between prep chunks (its group g needs
                # only the m-tiles prep chunk g//2 just produced), and the
                # prep PSUM tiles borrow the energy pool's slots so total
                # PSUM stays within 8 banks.
                NSUB = NCHW // 128
                NCH = NQ // NCHW
                NG = MT // MG
                import contextlib as _ctb

                with (
                    tc.tile_pool(name="ppool", bufs=4) as ppool,
                    tc.tile_pool(name="opool", bufs=4) as opool,
                    tc.tile_pool(name="zpool", bufs=6) as zpool,
                    _ctb.ExitStack() as _psum_stack,
                ):
                    _pools = {}

                    def open_po():
                        # 2 chunks of oT accumulators in flight (when they
                        # fit in PSUM next to the e tiles): the next chunk's
                        # PV matmuls start while the previous chunk's
                        # finalize is still reading its oT tiles.
                        po_bufs = 2 * NSUB if NSUB <= 2 else NSUB
                        _pools["po"] = _psum_stack.enter_context(
                            tc.tile_pool(name="po", bufs=po_bufs, space="PSUM")
                        )

                    def open_pe():
                        _pools["pe"] = _psum_stack.enter_context(
                            tc.tile_pool(name="pe", bufs=E_BUFS, space="PSUM")
                        )

                    def open_psum_pools():
                        open_pe()
                        open_po()

                    if _rep == 0:
                        vt_view = vt_s.rearrange("p (t w) -> p t w", w=VW)
                        mset_dt = BF16 if PV_BF16 else F32
                        nc.vector.memset(vt_view[:, :, C].bitcast(mset_dt), 1.0)
                        nc.vector.memset(vt_view[:, :, C + 1].bitcast(mset_dt), 0.0)

                    def _emit_v_tile(t, pool_v, v_tag):
                        ps = pool_v.tile(
                            [128, C], F32, name=f"r{_rep}psv{t}", tag=v_tag
                        )
                        nc.tensor.matmul(
                            ps[:],
                            xf_s[:, t * 128 : (t + 1) * 128],
                            w3t_s[0],
                            start=True,
                            stop=False,
                        )
                        nc.tensor.matmul(
                            ps[:],
                            xf_s[:, N + t * 128 : N + (t + 1) * 128],
                            w3t_s[1],
                            start=False,
                            stop=True,
                        )
                        nc.vector.tensor_copy(vt_s[:, t * VW : t * VW + C], ps[:])

                    def emit_proj_piece(pool, tag, w_s, b_s, dst, lo, w):
                        ps = pool.tile(
                            [128, w], F32, name=f"r{_rep}pp{lo}_{w}", tag=tag
                        )
                        nc.tensor.matmul(
                            ps[:], w_s[0], xf_s[:, lo : lo + w], start=True, stop=False
                        )
                        nc.tensor.matmul(
                            ps[:],
                            w_s[1],
                            xf_s[:, N + lo : N + lo + w],
                            start=False,
                            stop=True,
                        )
                        nc.vector.tensor_scalar_add(dst[:, lo : lo + w], ps[:], b_s)

                    def emit_K(ch, pool, tag):
                        emit_proj_piece(pool, tag, w2t_s, b2r_s, kp_s, ch * 512, 512)

                    def emit_Q(ch, pool, tag):
                        emit_proj_piece(pool, tag, w1t_s, b1r_s, qp_s, ch * 512, 512)

                    def emit_prep_chunk(ch, pool_kq, pool_v=None):
                        if pool_v is None:
                            pool_v = pool_kq
                        kq_tag = "e" if pool_kq is _pools.get("pe") else "psk"
                        v_tag = "e" if pool_v is _pools.get("pe") else "psv"
                        if ch == 0 and HEAD_SPLIT:
                            for q in range(2):
                                emit_proj_piece(
                                    pool_kq, kq_tag, w2t_s, b2r_s, kp_s, q * 256, 256
                                )
                            for q in range(2):
                                emit_proj_piece(
                                    pool_kq, kq_tag, w1t_s, b1r_s, qp_s, q * 256, 256
                                )
                            for t in range(4):
                                _emit_v_tile(t, pool_v, v_tag)
                            return
                        ps = pool_kq.tile(
                            [128, 512], F32, name=f"r{_rep}psk{ch}", tag=kq_tag
                        )
                        nc.tensor.matmul(
                            ps[:],
                            w2t_s[0],
                            xf_s[:, ch * 512 : (ch + 1) * 512],
                            start=True,
                            stop=False,
                        )
                        nc.tensor.matmul(
                            ps[:],
                            w2t_s[1],
                            xf_s[:, N + ch * 512 : N + (ch + 1) * 512],
                            start=False,
                            stop=True,
                        )
                        nc.vector.tensor_scalar_add(
                            kp_s[:, ch * 512 : (ch + 1) * 512], ps[:], b2r_s
                        )
                        if ch < NQ // 512:
                            ps = pool_kq.tile(
                                [128, 512], F32, name=f"r{_rep}psq{ch}", tag=kq_tag
                            )
                            nc.tensor.matmul(
                                ps[:],
                                w1t_s[0],
                                xf_s[:, ch * 512 : (ch + 1) * 512],
                                start=True,
                                stop=False,
                            )
                            nc.tensor.matmul(
                                ps[:],
                                w1t_s[1],
                                xf_s[:, N + ch * 512 : N + (ch + 1) * 512],
                                start=False,
                                stop=True,
                            )
                            nc.vector.tensor_scalar_add(
                                qp_s[:, ch * 512 : (ch + 1) * 512], ps[:], b1r_s
                            )
                        for t in range(ch * 4, ch * 4 + 4):
                            _emit_v_tile(t, pool_v, v_tag)

                    def emit_E(c, g, pool=None):
                        e = (pool or _pools["pe"]).tile(
                            [128, MG * NCHW], F32, name=f"r{_rep}e{c}_{g}", tag="e"
                        )
                        nbands = 4 // MG if E_BAND_ALT else 1
                        for i in range(MG):
                            mt = MG * g + i
                            band = (
                                32 * (MG * (g % nbands) + i) if E_PACK else 0
                            )
                            kw = dict(tile_position=(band, 0)) if E_PACK else {}
                            w_ap = kp_s[
                                band : band + 32, mt * 128 : (mt + 1) * 128
                            ]
                            if SPLIT_LDW and KDT is BF16:
                                nc.tensor.ldweights(w_ap, **kw)
                            mm = nc.tensor.matmul(
                                e[:, i * NCHW : (i + 1) * NCHW],
                                w_ap,
                                qp_s[band : band + 32, c * NCHW : (c + 1) * NCHW],
                                start=True,
                                stop=True,
                                **kw,
                            )
                            if SPLIT_LDW and KDT is BF16:
                                mm.ins.ldweights = False
                                _split_pairs.append(mm.ins.name)
                        return e

                    def make_oT(c):
                        return [
                            _pools["po"].tile(
                                [128, VW], F32, name=f"r{_rep}oT{c}_{s}", tag="oT"
                            )
                            for s in range(NSUB)
                        ]

                    def emit_group(c, g, e_cur, oT):
                        # exp(E - ESHIFT): softmax is shift-invariant; the
                        # constant rides the ACT instruction's free affine and
                        # moves the fp32 overflow ceiling from |E|~88 to ~113.
                        p = ppool.tile(
                            [128, MG * NCHW], PVDT, name=f"r{_rep}p{c}_{g}", tag="p"
                        )
                        if EXP_SPLIT:
                            h = MG * NCHW // 2
                            nc.scalar.activation(
                                p[:, 0:h], e_cur[:, 0:h], Exp, bias=eshift_s[:]
                            )
                            nc.scalar.activation(
                                p[:, h:], e_cur[:, h:], Exp, bias=eshift_s[:]
                            )
                        else:
                            nc.scalar.activation(
                                p[:], e_cur[:], Exp, bias=eshift_s[:]
                            )
                        for i in range(MG):
                            mt = MG * g + i
                            for s in range(NSUB):
                                nc.tensor.matmul(
                                    oT[s][:],
                                    p[:, i * NCHW + s * 128 : i * NCHW + (s + 1) * 128],
                                    vt_s[:, mt * VW : (mt + 1) * VW],
                                    start=(g == 0 and i == 0),
                                    stop=(g == NG - 1 and i == MG - 1),
                                )

                    def emit_finalize(c, oT, per_sub_dma=False):
                        otc = opool.tile(
                            [128, NSUB * C], ODT, name=f"r{_rep}otc{c}", tag="ot"
                        )
                        for s in range(NSUB):
                            ns = c * NSUB + s
                            zinv = zpool.tile(
                                [128, 1], F32, name=f"r{_rep}zinv{c}_{s}", tag="zinv"
                            )
                            nc.vector.reciprocal(zinv[:], oT[s][:, C : C + 1])
                            nc.vector.scalar_tensor_tensor(
                                otc[:, s * C : (s + 1) * C],
                                oT[s][:, 0:C],
                                zinv[:],
                                xqT_s[:, ns * C : (ns + 1) * C],
                                MULT,
                                ADD,
                            )
                            if per_sub_dma:
                                nc.sync.dma_start(
                                    outp[ns * 128 : (ns + 1) * 128, :],
                                    otc[:, s * C : (s + 1) * C],
                                )
                        if not per_sub_dma:
                            dst = outp[
                                c * NSUB * 128 : (c + 1) * NSUB * 128, :
                            ].rearrange("(s p) c -> p s c", p=128)
                            nc.sync.dma_start(
                                dst, otc.rearrange("p (s c) -> p s c", c=C)
                            )

                    if PREP_FUSE:
                        open_po()
                        oT = make_oT(0)
                        with (
                            tc.tile_pool(name="pp", bufs=1, space="PSUM") as pp,
                            tc.tile_pool(name="pv", bufs=1, space="PSUM") as pv,
                            tc.tile_pool(name="pe1", bufs=1, space="PSUM") as pe1,
                        ):
                            gpc = 4 // MG  # chunk-0 groups per prep chunk
                            for ch in range(N // 512):
                                emit_K(ch, pp, "psk")
                                if ch == 0:
                                    emit_Q(0, pp, "psk")
                                _emit_v_tile(4 * ch + 0, pv, "psv")
                                _emit_v_tile(4 * ch + 1, pv, "psv")
                                if gpc == 2:
                                    g = 2 * ch
                                    emit_group(0, g, emit_E(0, g, pe1), oT)
                                _emit_v_tile(4 * ch + 2, pv, "psv")
                                _emit_v_tile(4 * ch + 3, pv, "psv")
                                if gpc == 2:
                                    g = 2 * ch + 1
                                else:
                                    g = ch
                                emit_group(0, g, emit_E(0, g, pe1), oT)
                                if 0 < ch < NQ // 512:
                                    emit_Q(ch, pp, "psk")
                        emit_finalize(0, oT)
                        open_pe()
                        # phase 2: remaining chunks, software-pipelined
                        steps = [(c, g) for c in range(1, NCH) for g in range(NG)]
                        e_cur = emit_E(*steps[0])
                        for k, (c, g) in enumerate(steps):
                            if g == 0:
                                oT = make_oT(c)
                            e_next = (
                                emit_E(*steps[k + 1]) if k + 1 < len(steps) else None
                            )
                            emit_group(c, g, e_cur, oT)
                            e_cur = e_next
                            if g == NG - 1:
                                emit_finalize(c, oT, per_sub_dma=(c == NCH - 1))
                    else:
                        with (
                            tc.tile_pool(name="pp", bufs=2, space="PSUM") as pp,
                            tc.tile_pool(name="pv", bufs=2, space="PSUM") as pv,
                        ):
                            for ch in range(N // 512):
                                emit_prep_chunk(ch, pp, pv)
                        open_psum_pools()
                        steps = [(c, g) for c in range(NCH) for g in range(NG)]
                        e_cur = emit_E(*steps[0])
                        for k, (c, g) in enumerate(steps):
                            if g == 0:
                                oT = make_oT(c)
                            e_next = (
                                emit_E(*steps[k + 1]) if k + 1 < len(steps) else None
                            )
                            emit_group(c, g, e_cur, oT)
                            e_cur = e_next
                            if g == NG - 1:
                                emit_finalize(c, oT, per_sub_dma=(c == NCH - 1))

    _verify_split_ldw(nc, mybir, set(_split_pairs))
    _split_sync_waits(nc, mybir)
    return nc


def _verify_split_ldw(nc, mybir, pair_names):
    """A matmul with ldweights=False computes with whatever the PE array
    holds; ensure the immediately-preceding weight-touching PE instruction
    is its own InstLdweights (same weights AP). If the Tile scheduler
    moved anything in between, restore the self-loading mode."""
    if not pair_names:
        return
    refused = 0
    for fn in nc.m.functions:
        for blk in fn.blocks:
            last_w = None  # (kind, weights_ap_repr)
            for inst in blk.instructions:
                if isinstance(inst, mybir.InstLdweights):
                    last_w = ("ldw", repr(inst.ins[0]))
                elif isinstance(inst, mybir.InstMatmult):
                    if inst.name in pair_names:
                        ok = (
                            last_w is not None
                            and last_w[0] == "ldw"
                            and last_w[1] == repr(inst.ins[1])
                        )
                        if not ok:
                            inst.ldweights = True
                            refused += 1
                    last_w = ("mm", None)
    if refused:
        import logging

        logging.getLogger(__name__).warning(
            f"_verify_split_ldw: re-fused {refused} matmuls"
        )


def _get_graph(repeat=1, loop_n=1):
    key = f"nc{repeat}_{loop_n}"
    if key not in _CACHE:
        _CACHE[key] = _build_graph(repeat, loop_n)
    return _CACHE[key]


def _host_inputs(x, w1, b1, w2, b2, w3, b3):
    x = np.asarray(x, dtype=np.float32)
    xf = x.reshape(B, C, N)
    w1t = np.tile(np.asarray(w1, np.float32).T, (1, 4))  # [256, 128]
    w2t = np.tile(np.asarray(w2, np.float32).T, (1, 4))
    w3t = np.ascontiguousarray(np.asarray(w3, np.float32).T)  # [256, 256]
    wpack = np.concatenate([w1t, w2t, w3t], axis=1)  # [256, 512]
    if X_BF16:
        import ml_dtypes

        wpack = wpack.astype(ml_dtypes.bfloat16)
    bpack = np.stack(
        [np.tile(np.asarray(b1, np.float32), 4), np.tile(np.asarray(b2, np.float32), 4)],
        axis=1,
    )  # [128, 2]
    b3f = np.asarray(b3, np.float32)

    in_maps = []
    for core in range(N_CORES):
        b, half = divmod(core, 2)
        n0 = half * NQ
        xb = xf[b]
        # roll so this core's query range sits at columns 0:NQ; K/V sums
        # over m are permutation-invariant so the roll is harmless.
        x_roll = np.ascontiguousarray(np.roll(xb, -n0, axis=1))
        if X_BF16:
            import ml_dtypes

            x_roll = x_roll.astype(ml_dtypes.bfloat16)
        xqTc = np.ascontiguousarray(xb[:, n0 : n0 + NQ].T + b3f[None, :])
        in_maps.append(
            {"xf": x_roll, "xqT": xqTc, "wpack": wpack, "bpack": bpack}
        )
    return in_maps


def kernel(x, w1, b1, w2, b2, w3, b3, _trace=False, _repeat=1):
    import sys

    if "/opt/trn_rl_repo" not in sys.path:
        sys.path.insert(0, "/opt/trn_rl_repo")
    from concourse.bass_utils import run_bass_kernel_spmd

    nc = _get_graph(_repeat)
    in_maps = _host_inputs(x, w1, b1, w2, b2, w3, b3)
    res = run_bass_kernel_spmd(nc, in_maps, list(range(N_CORES)), trace=_trace)
    _CACHE["last_result"] = res

    out = np.empty((B, C, N), np.float32)
    for core in range(N_CORES):
        b, half = divmod(core, 2)
        n0 = half * NQ
        out[b][:, n0 : n0 + NQ] = res.results[core]["out"].astype(np.float32).T
    return out.reshape(B, C, 64, 64)



# revision 35
# speedup vs baseline: 9.0031x; 7.4092x over previous
"""
Self-attention (SAGAN-style) Bass kernel for Trainium2, 8 NeuronCores.

Reference computation (per batch b, X = x[b] reshaped [C=256, N=4096]):
    q = W1 X + b1          [D=32, N]
    k = W2 X + b2          [D=32, N]
    v = W3 X + b3          [C, N]
    E[n, m]   = q[:, n] . k[:, m]
    A         = softmax over m of E
    out[c, n] = sum_m v[c, m] A[n, m]  + X[c, n]

Sharding: 8 cores = 4 batches x 2 query-halves. Each core gets the full
X for its batch (needed for K/V over all N) but computes only its half of
the query rows -> no inter-core communication at all.

Device algorithm (per core, NQ = 2048 query columns):
  - Q' = W1 X[:, 0:NQ], K' = W2 X, each replicated 4x along partitions
    (host pre-tiles the weights) so the K=32 energy matmuls can use
    PE row-tiling.
  - V_T[m, c] = (X^T W3^T)[m, c] computed natively (no transposes), with
    an appended constant ones column: V_aug = [V_T | 1 | 0pad]. The V bias
    b3 drops out of the matmuls entirely: its contribution to out_T is
    Z[n]*b3[c], which after the /Z normalization is just +b3 -- folded
    into the host-prepared residual xqT.
  - E^T[m, n] = K'^T Q' with m on partitions -> exp(E - 25) without any
    per-row max subtraction (softmax is shift-invariant; the constant
    rides the ACT affine and keeps fp32 safe up to |E| ~ 113) -> P^T. The two K=32 energy
    matmuls of each group run concurrently in separate PE row-bands
    (tile_position), and exp is split in two so PV can start after the
    first half.
  - out_T[n, c], Z[n] = P^T.T @ V_aug accumulated over all m in PSUM:
    the ones column yields the softmax denominator for free.
  - out = out_T[:, 0:256] * (1/Z) + (X_q^T + b3)  (one fused vector op),
    DMA out in [n, c] layout; the host transposes back when unsharding.

All matmul operands are stored as bf16 (x, weights, Q', K', P, V_aug):
the PE streams 1 column/cycle either way, but bf16 halves the
LDWEIGHTS cost via FWL (fast weight load) — and on this toolchain the
per-matmul weight load is fully serialized with the matmul (measured
156 ns for LDW+MM at 258 free-dim), so LDW time is on the critical
path everywhere. PSUM accumulation stays fp32; the residual/normalize
path (xqT, biases, 1/Z) stays fp32. Energy groups alternate PE row
bands (g even: rows 0-63, g odd: 64-127) so consecutive groups'
LDWEIGHTS can pull ahead and their matmuls run concurrently.
Measured on HW (test.py loop-slope): 150278 ns (fp32r baseline)
-> 134466 (bf16 operands, merged exp, band alternation)
-> 132468 (fused prep, bf16 x/weights)
-> 128378 (For_i staggered_reset + PE branch-prefetch hints;
   cold-process spread 128-137, warm in-process 122-127)
-> 120642 -> 112546 (same kernel, spike-robust measurement in
   test.py and a clean device window: the axon RPC adds random ~+39ms
   spikes to ~40% of calls which corrupted the median slope; the
   guarded estimator recovers the clean value).
kernel_v7.py (phase-1 prep evacuations through one shared 2-slot
PSUM pool + bf16 output DMA) was REJECTED: bracketed paired-median
A/B measured it ~8us slower -- cycling all 7-8 prep allocations per
chunk through 2 shared slots serializes phase-1 worse than the
split single-buffered pp/pv pools.
PE-busy floor model: PV 512x(107.5 stream + 53 LDW + 4 NX) = 84us,
energy ~14.5us (2-band concurrent), prep ~17us -> ~119us; the wall
is PE-bound, ACT exp is ~77us busy and off the critical path.
"""

import numpy as np

E_PACK = True  # row-tile the K=32 energy matmuls via tile_position
HEAD_SPLIT = True  # 256-col first chunk: first matmul starts earlier
PREP_FUSE = True  # weave chunk-0 attention into the prep phase
OUT_ONE_DMA = True  # one output DMA per n-chunk instead of per n-sub
E_BF16 = True  # Q'/K' in bf16: FWL fast weight loads on energy matmuls
VT_STT = True  # V bias via one DVE scalar_tensor_tensor instead of K=1 matmuls
EXP_SPLIT = False  # two ACT instructions per group: PV can start after the first half
ABL_EXP_HALF = False  # timing ablation: exp over half the tile (WRONG results)
ABL_PV_HALF = False  # timing ablation: skip half the PV matmuls (WRONG results)
MG = 2  # m-tiles per exp group (<=4; MG concurrent PE row-bands). MG=4 with
# NCHW=256 is ILLEGAL: the band matmuls would write 256-col halves of the
# same PSUM bank concurrently -> fatal bank write collision on HW.
NCHW = 512  # n-chunk width (columns of Q processed per oT accumulation pass)
E_BUFS = 2  # psum double/triple buffering for energy tiles
PV_BF16 = True  # store P and V_aug as bf16: FWL fast weight load on PV matmuls
E_BAND_ALT = True  # alternate PE row-bands across energy groups: LDW overlaps
SPLIT_LDW = False  # bacc already auto-splits LDW from matmuls; extras load twice
X_BF16 = True  # xf/wpack in bf16: FWL on V-prep/KQ-proj stationaries, half DMA
LOOP_HINTS = True  # For_i branch prefetch hints (body >256 PE instructions)
OUT_BF16 = False  # bf16 output DMA: halves out traffic, shorter tail

B = 4
C = 256
D = 32
N = 4096
NQ = N // 2  # query columns per core
N_CORES = 8

_CACHE = {}


def _patch_tile_drain(tile_mod, mybir):
    """Walrus in this container rejects >2 sync waits on a Drain
    instruction ("Too many sync wait commands"). Split the TileContext
    tail-drain waits into single-wait NOPs instead."""
    if getattr(tile_mod.TileContext, "_drain_patched", False):
        return

    def _drain_and_barrier(self, tick_clock, wait_clock):
        from concourse.tile import ScopedClock

        drain_inst = self.nc.sync.drain()
        wait_clock.add_sem_waits(
            drain_inst.ins, ScopedClock({None: tick_clock.global_clock})
        )
        si = drain_inst.ins.sync_info
        if si is not None and len(si.on_wait) > 1:
            waits = list(si.on_wait)
            # Engine-owned sems are re-synchronized by the all_engine_barrier
            # below; only DMA-queue sems (no engine) must be awaited here.
            dma_waits = [w for w in waits if "DMA" in (w.ant_name or "")]
            eng_waits = [w for w in waits if "DMA" not in (w.ant_name or "")]
            keep = dma_waits if dma_waits else waits
            drain_inst.ins.sync_info = mybir.SyncInfo(
                on_wait=[keep[0]], on_update=[]
            )
            for w in keep[1:]:
                n = self.nc.sync.nop()
                n.ins.sync_info = mybir.SyncInfo(on_wait=[w], on_update=[])
        self.nc.all_engine_barrier()
        popped = self.nc._tile_sem_poison_stack.pop()
        assert popped is self._sem_poison
        self.nc.clear_and_free_semaphores(list(self.sems.allocated().values()))
        self.nc.all_engine_barrier()

    tile_mod.TileContext._drain_and_barrier = _drain_and_barrier
    tile_mod.TileContext._drain_patched = True


def _split_sync_waits(nc, mybir, max_waits=1):
    """This container's walrus rejects instructions carrying more than
    ~1 sync wait (e.g. Matmult/S3_LW, Drain/CTRL). Hoist extra waits onto
    same-engine NOPs placed immediately before the instruction."""
    ctr = [0]
    for fn in nc.m.functions:
        for blk in fn.blocks:
            new_insts = []
            for inst in blk.instructions:
                si = inst.sync_info
                if si is not None and len(si.on_wait) > max_waits:
                    waits = list(si.on_wait)
                    for w in waits[max_waits:]:
                        ctr[0] += 1
                        nop = mybir.InstNoOp(
                            name=f"WSPLIT-{ctr[0]}",
                            sync_info=mybir.SyncInfo(on_wait=[w], on_update=[]),
                            bass_nofuse=True,
                            engine=inst.engine,
                        )
                        new_insts.append(nop)
                    inst.sync_info = mybir.SyncInfo(
                        on_wait=waits[:max_waits], on_update=list(si.on_update)
                    )
                new_insts.append(inst)
            blk.instructions = new_insts


def _build_graph(repeat=1, loop_n=1):
    import sys

    if "/opt/trn_rl_repo" not in sys.path:
        sys.path.insert(0, "/opt/trn_rl_repo")
    import concourse.bass as bass
    import concourse.mybir as mybir
    import concourse.tile as tile

    _patch_tile_drain(tile, mybir)

    F32 = mybir.dt.float32
    F32R = mybir.dt.float32r
    BF16 = mybir.dt.bfloat16
    PVDT = BF16 if PV_BF16 else F32R
    EDT = BF16 if E_BF16 in (True, 1) else F32R
    KDT = BF16 if E_BF16 in (True, 1, 2) else F32R
    XDT = BF16 if X_BF16 else F32R
    Exp = mybir.ActivationFunctionType.Exp
    MULT = mybir.AluOpType.mult
    ADD = mybir.AluOpType.add

    _split_pairs = []  # names of InstMatmult with ldweights=False

    nc = bass.Bass()
    xf = nc.declare_dram_parameter("xf", [C, N], XDT, isOutput=False)
    xqT = nc.declare_dram_parameter("xqT", [NQ, C], F32, isOutput=False)
    wpack = nc.declare_dram_parameter("wpack", [C, 512], XDT, isOutput=False)
    bpack = nc.declare_dram_parameter("bpack", [128, 2], F32, isOutput=False)
    ODT = BF16 if OUT_BF16 else F32
    outp = nc.declare_dram_parameter("out", [NQ, C], ODT, isOutput=True)

    MT = N // 128  # 32 m-tiles
    VW = C + 2  # 258: V_T plus ones column plus pad (fp32r needs even free dim)

    with tile.TileContext(nc) as tc:
        with tc.tile_pool(name="consts", bufs=1) as consts:
            # packed weights/biases first (few DMAs) so projections start fast
            # wp_s layout per c-half h (rows h*128..): [w1t | w2t | w3t] cols
            wp_s = consts.tile([128, 1024], XDT)
            nc.sync.dma_start(wp_s[:, 0:512], wpack[0:128, :])
            nc.sync.dma_start(wp_s[:, 512:1024], wpack[128:256, :])
            bp_s = consts.tile([128, 2], F32)
            w1t_s = [wp_s[:, 0:128], wp_s[:, 512:640]]  # per c-half
            w2t_s = [wp_s[:, 128:256], wp_s[:, 640:768]]
            w3t_s = [wp_s[:, 256:512], wp_s[:, 768:1024]]
            b1r_s = bp_s[:, 0:1]
            b2r_s = bp_s[:, 1:2]
            eshift_s = consts.tile([128, 1], F32)
            nc.vector.memset(eshift_s[:], -25.0)
            # xf in interleaved half-pairs so chunk-ch projections unblock
            # as soon as their slices land
            xf_s = consts.tile([128, 2 * N], XDT)
            if HEAD_SPLIT:
                for q in range(2):  # chunk 0 in 256-col quarters
                    nc.sync.dma_start(
                        xf_s[:, q * 256 : (q + 1) * 256],
                        xf[0:128, q * 256 : (q + 1) * 256],
                    )
                    nc.sync.dma_start(
                        xf_s[:, N + q * 256 : N + (q + 1) * 256],
                        xf[128:256, q * 256 : (q + 1) * 256],
                    )
                    if q == 0:
                        nc.sync.dma_start(bp_s[:], bpack[:])
            else:
                nc.sync.dma_start(bp_s[:], bpack[:])
            for ch in range(0 if not HEAD_SPLIT else 1, 8):
                nc.sync.dma_start(
                    xf_s[:, ch * 512 : (ch + 1) * 512],
                    xf[0:128, ch * 512 : (ch + 1) * 512],
                )
                nc.sync.dma_start(
                    xf_s[:, N + ch * 512 : N + (ch + 1) * 512],
                    xf[128:256, ch * 512 : (ch + 1) * 512],
                )
            xqT_s = consts.tile([128, (NQ // 128) * C], F32)
            for s in range(NQ // 128):
                nc.sync.dma_start(
                    xqT_s[:, s * C : (s + 1) * C], xqT[s * 128 : (s + 1) * 128, :]
                )

            qp_s = consts.tile([128, NQ], EDT)  # Q' replicated x4 on partitions
            kp_s = consts.tile([128, N], KDT)  # K' replicated x4
            vt_s = consts.tile([128, MT * VW], PVDT)  # [V_T | 1 | 0] per m-tile

            import contextlib

            loop_kw = (
                dict(
                    hint_engines=(mybir.EngineType.PE,),
                    staggered_reset=True,
                )
                if LOOP_HINTS
                else {}
            )
            loop_ctx = (
                tc.For_i(0, loop_n, 1, **loop_kw)
                if loop_n > 1
                else contextlib.nullcontext()
            )
            with loop_ctx:
              for _rep in range(repeat):
                # ---- compute body ----
                # Prep (K'/Q'/V_T projections) is emitted per 512-col chunk so
                # it tracks the xf DMA stream. With PREP_FUSE, chunk-0 of the
                # attention is woven between prep chunks (its group g needs
                # only the m-tiles prep chunk g//2 just produced), and the
                # prep PSUM tiles borrow the energy pool's slots so total
                # PSUM stays within 8 banks.
                NSUB = NCHW // 128
                NCH = NQ // NCHW
                NG = MT // MG
                import contextlib as _ctb

                with (
                    tc.tile_pool(name="ppool", bufs=4) as ppool,
                    tc.tile_pool(name="opool", bufs=4) as opool,
                    tc.tile_pool(name="zpool", bufs=6) as zpool,
                    _ctb.ExitStack() as _psum_stack,
                ):
                    _pools = {}

                    def open_po():
                        # 2 chunks of oT accumulators in flight (when they
                        # fit in PSUM next to the e tiles): the next chunk's
                        # PV matmuls start while the previous chunk's
                        # finalize is still reading its oT tiles.
                        po_bufs = 2 * NSUB if NSUB <= 2 else NSUB
                        _pools["po"] = _psum_stack.enter_context(
                            tc.tile_pool(name="po", bufs=po_bufs, space="PSUM")
                        )

                    def open_pe():
                        _pools["pe"] = _psum_stack.enter_context(
                            tc.tile_pool(name="pe", bufs=E_BUFS, space="PSUM")
                        )

                    def open_psum_pools():
                        open_pe()
                        open_po()

                    if _rep == 0:
                        vt_view = vt_s.rearrange("p (t w) -> p t w", w=VW)
                        mset_dt = BF16 if PV_BF16 else F32
                        nc.vector.memset(vt_view[:, :, C].bitcast(mset_dt), 1.0)
                        nc.vector.memset(vt_view[:, :, C + 1].bitcast(mset_dt), 0.0)

                    def _emit_v_tile(t, pool_v, v_tag):
                        ps = pool_v.tile(
                            [128, C], F32, name=f"r{_rep}psv{t}", tag=v_tag
                        )
                        nc.tensor.matmul(
                            ps[:],
                            xf_s[:, t * 128 : (t + 1) * 128],
                            w3t_s[0],
                            start=True,
                            stop=False,
                        )
                        nc.tensor.matmul(
                            ps[:],
                            xf_s[:, N + t * 128 : N + (t + 1) * 128],
                            w3t_s[1],
                            start=False,
                            stop=True,
                        )
                        nc.vector.tensor_copy(vt_s[:, t * VW : t * VW + C], ps[:])

                    def emit_proj_piece(pool, tag, w_s, b_s, dst, lo, w):
                        ps = pool.tile(
                            [128, w], F32, name=f"r{_rep}pp{lo}_{w}", tag=tag
                        )
                        nc.tensor.matmul(
                            ps[:], w_s[0], xf_s[:, lo : lo + w], start=True, stop=False
                        )
                        nc.tensor.matmul(
                            ps[:],
                            w_s[1],
                            xf_s[:, N + lo : N + lo + w],
                            start=False,
                            stop=True,
                        )
                        nc.vector.tensor_scalar_add(dst[:, lo : lo + w], ps[:], b_s)

                    def emit_K(ch, pool, tag):
                        emit_proj_piece(pool, tag, w2t_s, b2r_s, kp_s, ch * 512, 512)

                    def emit_Q(ch, pool, tag):
                        emit_proj_piece(pool, tag, w1t_s, b1r_s, qp_s, ch * 512, 512)

                    def emit_prep_chunk(ch, pool_kq, pool_v=None):
                        if pool_v is None:
                            pool_v = pool_kq
                        kq_tag = "e" if pool_kq is _pools.get("pe") else "psk"
                        v_tag = "e" if pool_v is _pools.get("pe") else "psv"
                        if ch == 0 and HEAD_SPLIT:
                            for q in range(2):
                                emit_proj_piece(
                                    pool_kq, kq_tag, w2t_s, b2r_s, kp_s, q * 256, 256
                                )
                            for q in range(2):
                                emit_proj_piece(
                                    pool_kq, kq_tag, w1t_s, b1r_s, qp_s, q * 256, 256
                                )
                            for t in range(4):
                                _emit_v_tile(t, pool_v, v_tag)
                            return
                        ps = pool_kq.tile(
                            [128, 512], F32, name=f"r{_rep}psk{ch}", tag=kq_tag
                        )
                        nc.tensor.matmul(
                            ps[:],
                            w2t_s[0],
                            xf_s[:, ch * 512 : (ch + 1) * 512],
                            start=True,
                            stop=False,
                        )
                        nc.tensor.matmul(
                            ps[:],
                            w2t_s[1],
                            xf_s[:, N + ch * 512 : N + (ch + 1) * 512],
                            start=False,
                            stop=True,
                        )
                        nc.vector.tensor_scalar_add(
                            kp_s[:, ch * 512 : (ch + 1) * 512], ps[:], b2r_s
                        )
                        if ch < NQ // 512:
                            ps = pool_kq.tile(
                                [128, 512], F32, name=f"r{_rep}psq{ch}", tag=kq_tag
                            )
                            nc.tensor.matmul(
                                ps[:],
                                w1t_s[0],
                                xf_s[:, ch * 512 : (ch + 1) * 512],
                                start=True,
                                stop=False,
                            )
                            nc.tensor.matmul(
                                ps[:],
                                w1t_s[1],
                                xf_s[:, N + ch * 512 : N + (ch + 1) * 512],
                                start=False,
                                stop=True,
                            )
                            nc.vector.tensor_scalar_add(
                                qp_s[:, ch * 512 : (ch + 1) * 512], ps[:], b1r_s
                            )
                        for t in range(ch * 4, ch * 4 + 4):
                            _emit_v_tile(t, pool_v, v_tag)

                    def emit_E(c, g, pool=None):
                        e = (pool or _pools["pe"]).tile(
                            [128, MG * NCHW], F32, name=f"r{_rep}e{c}_{g}", tag="e"
                        )
                        nbands = 4 // MG if E_BAND_ALT else 1
                        for i in range(MG):
                            mt = MG * g + i
                            band = (
                                32 * (MG * (g % nbands) + i) if E_PACK else 0
                            )
                            kw = dict(tile_position=(band, 0)) if E_PACK else {}
                            w_ap = kp_s[
                                band : band + 32, mt * 128 : (mt + 1) * 128
                            ]
                            if SPLIT_LDW and KDT is BF16:
                                nc.tensor.ldweights(w_ap, **kw)
                            mm = nc.tensor.matmul(
                                e[:, i * NCHW : (i + 1) * NCHW],
                                w_ap,
                                qp_s[band : band + 32, c * NCHW : (c + 1) * NCHW],
                                start=True,
                                stop=True,
                                **kw,
                            )
                            if SPLIT_LDW and KDT is BF16:
                                mm.ins.ldweights = False
                                _split_pairs.append(mm.ins.name)
                        return e

                    def make_oT(c):
                        return [
                            _pools["po"].tile(
                                [128, VW], F32, name=f"r{_rep}oT{c}_{s}", tag="oT"
                            )
                            for s in range(NSUB)
                        ]

                    def emit_group(c, g, e_cur, oT):
                        # exp(E - ESHIFT): softmax is shift-invariant; the
                        # constant rides the ACT instruction's free affine and
                        # moves the fp32 overflow ceiling from |E|~88 to ~113.
                        p = ppool.tile(
                            [128, MG * NCHW], PVDT, name=f"r{_rep}p{c}_{g}", tag="p"
                        )
                        if EXP_SPLIT:
                            h = MG * NCHW // 2
                            nc.scalar.activation(
                                p[:, 0:h], e_cur[:, 0:h], Exp, bias=eshift_s[:]
                            )
                            nc.scalar.activation(
                                p[:, h:], e_cur[:, h:], Exp, bias=eshift_s[:]
                            )
                        else:
                            nc.scalar.activation(
                                p[:], e_cur[:], Exp, bias=eshift_s[:]
                            )
                        for i in range(MG):
                            mt = MG * g + i
                            for s in range(NSUB):
                                nc.tensor.matmul(
                                    oT[s][:],
                                    p[:, i * NCHW + s * 128 : i * NCHW + (s + 1) * 128],
                                    vt_s[:, mt * VW : (mt + 1) * VW],
                                    start=(g == 0 and i == 0),
                                    stop=(g == NG - 1 and i == MG - 1),
                                )

                    def emit_finalize(c, oT, per_sub_dma=False):
                        otc = opool.tile(
                            [128, NSUB * C], ODT, name=f"r{_rep}otc{c}", tag="ot"
                        )
                        for s in range(NSUB):
                            ns = c * NSUB + s
                            zinv = zpool.tile(
                                [128, 1], F32, name=f"r{_rep}zinv{c}_{s}", tag="zinv"
                            )
                            nc.vector.reciprocal(zinv[:], oT[s][:, C : C + 1])
                            nc.vector.scalar_tensor_tensor(
                                otc[:, s * C : (s + 1) * C],
                                oT[s][:, 0:C],
                                zinv[:],
                                xqT_s[:, ns * C : (ns + 1) * C],
                                MULT,
                                ADD,
                            )
                            if per_sub_dma:
                                nc.sync.dma_start(
                                    outp[ns * 128 : (ns + 1) * 128, :],
                                    otc[:, s * C : (s + 1) * C],
                                )
                        if not per_sub_dma:
                            dst = outp[
                                c * NSUB * 128 : (c + 1) * NSUB * 128, :
                            ].rearrange("(s p) c -> p s c", p=128)
                            nc.sync.dma_start(
                                dst, otc.rearrange("p (s c) -> p s c", c=C)
                            )

                    if PREP_FUSE:
                        open_po()
                        oT = make_oT(0)
                        with (
                            tc.tile_pool(name="pp", bufs=1, space="PSUM") as pp,
                            tc.tile_pool(name="pv", bufs=1, space="PSUM") as pv,
                            tc.tile_pool(name="pe1", bufs=1, space="PSUM") as pe1,
                        ):
                            gpc = 4 // MG  # chunk-0 groups per prep chunk
                            for ch in range(N // 512):
                                emit_K(ch, pp, "psk")
                                if ch == 0:
                                    emit_Q(0, pp, "psk")
                                _emit_v_tile(4 * ch + 0, pv, "psv")
                                _emit_v_tile(4 * ch + 1, pv, "psv")
                                if gpc == 2:
                                    g = 2 * ch
                                    emit_group(0, g, emit_E(0, g, pe1), oT)
                                _emit_v_tile(4 * ch + 2, pv, "psv")
                                _emit_v_tile(4 * ch + 3, pv, "psv")
                                if gpc == 2:
                                    g = 2 * ch + 1
                                else:
                                    g = ch
                                emit_group(0, g, emit_E(0, g, pe1), oT)
                                if 0 < ch < NQ // 512:
                                    emit_Q(ch, pp, "psk")
                        emit_finalize(0, oT)
                        open_pe()
                        # phase 2: remaining chunks, software-pipelined
                        steps = [(c, g) for c in range(1, NCH) for g in range(NG)]
                        e_cur = emit_E(*steps[0])
                        for k, (c, g) in enumerate(steps):
                            if g == 0:
                                oT = make_oT(c)
                            e_next = (
                                emit_E(*steps[k + 1]) if k + 1 < len(steps) else None
                            )
                            emit_group(c, g, e_cur, oT)
                            e_cur = e_next
                            if g == NG - 1:
                                emit_finalize(c, oT, per_sub_dma=(c == NCH - 1))
                    else:
                        with (
                            tc.tile_pool(name="pp", bufs=2, space="PSUM") as pp,
                            tc.tile_pool(name="pv", bufs=2, space="PSUM") as pv,
                        ):
                            for ch in range(N // 512):
                                emit_prep_chunk(ch, pp, pv)
                        open_psum_pools()
                        steps = [(c, g) for c in range(NCH) for g in range(NG)]
                        e_cur = emit_E(*steps[0])
                        for k, (c, g) in enumerate(steps):
                            if g == 0:
                                oT = make_oT(c)
                            e_next = (
                                emit_E(*steps[k + 1]) if k + 1 < len(steps) else None
                            )
                            emit_group(c, g, e_cur, oT)
                            e_cur = e_next
                            if g == NG - 1:
                                emit_finalize(c, oT, per_sub_dma=(c == NCH - 1))

    _verify_split_ldw(nc, mybir, set(_split_pairs))
    _split_sync_waits(nc, mybir)
    return nc


def _verify_split_ldw(nc, mybir, pair_names):
    """A matmul with ldweights=False computes with whatever the PE array
    holds; ensure the immediately-preceding weight-touching PE instruction
    is its own InstLdweights (same weights AP). If the Tile scheduler
    moved anything in between, restore the self-loading mode."""
    if not pair_names:
        return
    refused = 0
    for fn in nc.m.functions:
        for blk in fn.blocks:
            last_w = None  # (kind, weights_ap_repr)
            for inst in blk.instructions:
                if isinstance(inst, mybir.InstLdweights):
                    last_w = ("ldw", repr(inst.ins[0]))
                elif isinstance(inst, mybir.InstMatmult):
                    if inst.name in pair_names:
                        ok = (
                            last_w is not None
                            and last_w[0] == "ldw"
                            and last_w[1] == repr(inst.ins[1])
                        )
                        if not ok:
                            inst.ldweights = True
                            refused += 1
                    last_w = ("mm", None)
    if refused:
        import logging

        logging.getLogger(__name__).warning(
            f"_verify_split_ldw: re-fused {refused} matmuls"
        )


def _get_graph(repeat=1, loop_n=1):
    key = f"nc{repeat}_{loop_n}"
    if key not in _CACHE:
        _CACHE[key] = _build_graph(repeat, loop_n)
    return _CACHE[key]


def _host_inputs(x, w1, b1, w2, b2, w3, b3):
    x = np.asarray(x, dtype=np.float32)
    xf = x.reshape(B, C, N)
    w1t = np.tile(np.asarray(w1, np.float32).T, (1, 4))  # [256, 128]
    w2t = np.tile(np.asarray(w2, np.float32).T, (1, 4))
    w3t = np.ascontiguousarray(np.asarray(w3, np.float32).T)  # [256, 256]
    wpack = np.concatenate([w1t, w2t, w3t], axis=1)  # [256, 512]
    if X_BF16:
        import ml_dtypes

        wpack = wpack.astype(ml_dtypes.bfloat16)
    bpack = np.stack(
        [np.tile(np.asarray(b1, np.float32), 4), np.tile(np.asarray(b2, np.float32), 4)],
        axis=1,
    )  # [128, 2]
    b3f = np.asarray(b3, np.float32)

    in_maps = []
    for core in range(N_CORES):
        b, half = divmod(core, 2)
        n0 = half * NQ
        xb = xf[b]
        # roll so this core's query range sits at columns 0:NQ; K/V sums
        # over m are permutation-invariant so the roll is harmless.
        x_roll = np.ascontiguousarray(np.roll(xb, -n0, axis=1))
        if X_BF16:
            import ml_dtypes

            x_roll = x_roll.astype(ml_dtypes.bfloat16)
        xqTc = np.ascontiguousarray(xb[:, n0 : n0 + NQ].T + b3f[None, :])
        in_maps.append(
            {"xf": x_roll, "xqT": xqTc, "wpack": wpack, "bpack": bpack}
        )
    return in_maps


def kernel(x, w1, b1, w2, b2, w3, b3, _trace=False, _repeat=1):
    import sys

    if "/opt/trn_rl_repo" not in sys.path:
        sys.path.insert(0, "/opt/trn_rl_repo")
    from concourse.bass_utils import run_bass_kernel_spmd

    nc = _get_graph(_repeat)
    in_maps = _host_inputs(x, w1, b1, w2, b2, w3, b3)
    res = run_bass_kernel_spmd(nc, in_maps, list(range(N_CORES)), trace=_trace)
    _CACHE["last_result"] = res

    out = np.empty((B, C, N), np.float32)
    for core in range(N_CORES):
        b, half = divmod(core, 2)
        n0 = half * NQ
        out[b][:, n0 : n0 + NQ] = res.results[core]["out"].astype(np.float32).T
    return out.reshape(B, C, 64, 64)

